# revision 1
# baseline (speedup 1.0000x reference)
"""Low-rank attention Trainium2 kernel (8 NeuronCores, SPMD).

Math (reference):
    tmp = relu(x @ W.T + b); U,V,Z,T = split(tmp, 4, axis=1)
    norm = sum(U @ colsum(V)) / n + eps ;  D = 1/norm
    out = concat[(U @ (V.T @ Z)) * D, T]

Sharding: rows of x across 8 cores. Per-core partials (V.T@Z [k,k],
colsum(V), colsum(U)) are AllReduced on-device; each core then computes
its local U @ (VtZ) * D.

Design notes (measured on trn2, ~338us vs 409us fp32r baseline):
- bf16 matmul operands, fp32 PSUM accumulation (~1.8e-3 rel err vs the
  2e-2 gate; PE streams ~2 rows/ns under the board power throttle).
- x^T fully resident in SBUF (16 KB/partition per d-tile x 8): the whole
  T-pass defers behind the AllReduce with zero HBM reloads.
- Split AllReduce: VtZ partial for ibs [0,12) reduces mid-phase-1 (absorbs
  the ~15-25us inter-core rendezvous while the DMA fabric is idle); the
  remainder + csu reduce at phase-1 end, hidden by the deferred T-pass.
- colsum(V) rides as a ones-column appended to Z inside the V^T@Z matmul.
- The deferred T matmuls read a gated copy of the T-weight columns (gate
  derived from csu), a true data dependency that stops the Tile scheduler
  from hoisting them out of the AllReduce window.
- Output DMAs batched 2 row-tiles per descriptor; PSUM->SBUF drains split
  across DVE and ACT.
- Phase 4 issues h-major over groups of 4 PSUM tiles (moving operand fixed
  across the group, start/stop pairs spread apart): 2-mm groups with
  alternating moving operands measured at HALF the PE issue rate.
"""
import sys

sys.path.insert(0, "/opt/trn_rl_repo")
import numpy as np
import ml_dtypes

BF16 = ml_dtypes.bfloat16

NCORES = 8
N_ROWS, D_IN, K = 65536, 1024, 256
NLOC = N_ROWS // NCORES      # 8192 rows per core
P = 128
IB = 512                     # i-block width
NB = NLOC // IB              # 16 blocks
EPS = 1e-6
TDEF = 16                    # T-pass blocks deferred to overlap the AllReduce
XCHUNKS = [(0, 512), (512, 512), (1024, 1024), (2048, 2048), (4096, 2048), (6144, 2048)]

_built = {}


def _build(d_rows):
    import concourse.bacc as bacc
    import concourse.mybir as mybir
    import concourse.tile as tile

    dt = mybir.dt
    f32, bf16 = dt.float32, dt.bfloat16
    RELU = mybir.ActivationFunctionType.Relu
    DT = d_rows // P
    NSUB = IB // P

    nc = bacc.Bacc("TRN2", target_bir_lowering=False, debug=False, num_devices=NCORES)
    xT = nc.dram_tensor("xT", [d_rows, NLOC], bf16, kind="ExternalInput")
    WT = nc.dram_tensor("WT", [d_rows, 4 * K], bf16, kind="ExternalInput")
    out = nc.dram_tensor("out", [NLOC, 2 * K], f32, kind="ExternalOutput")

    with tile.TileContext(nc) as tc:
        with (
            tc.tile_pool(name="wp", bufs=1) as wp,
            tc.tile_pool(name="xp", bufs=1) as xp,
            tc.tile_pool(name="up", bufs=1) as up,
            tc.tile_pool(name="vzp", bufs=5) as vzp,
            tc.tile_pool(name="ob", bufs=6) as ob,
            tc.tile_pool(name="acc", bufs=1) as accp,
            tc.tile_pool(name="ps", bufs=8, space="PSUM") as ps,
            tc.tile_pool(name="dram", bufs=1, space="DRAM") as dram,
        ):
            # Weights (split gpsimd/scalar queues) and resident x^T (sync queue,
            # kd-interleaved chunks, small first chunks so ib0 can start early).
            # W loads split column-wise: U-cols first (ib0's first matmuls),
            # V|Z next; the T-cols are not needed until the deferred T-pass
            # (phase-1 end), so they leave the critical priming bandwidth and
            # issue after the early x chunks.
            wt = []
            for kd in range(DT):
                w = wp.tile([P, 4 * K], bf16, tag=f"w{kd}", name=f"w{kd}")
                nc.sync.dma_start(
                    out=w[:, 0:K], in_=WT[kd * P:(kd + 1) * P, 0:K])
                q = nc.gpsimd if kd < DT // 2 else nc.scalar
                q.dma_start(
                    out=w[:, K:3 * K], in_=WT[kd * P:(kd + 1) * P, K:3 * K])
                wt.append(w)
            xf = [xp.tile([P, NLOC], bf16, tag=f"x{kd}", name=f"x{kd}") for kd in range(DT)]
            for ci, (c0, cw) in enumerate(XCHUNKS):
                for kd in range(DT):
                    nc.sync.dma_start(
                        out=xf[kd][:, c0:c0 + cw],
                        in_=xT[kd * P:(kd + 1) * P, c0:c0 + cw],
                    )
                if ci == 2:
                    for kd in range(DT):
                        nc.sync.dma_start(
                            out=wt[kd][:, 3 * K:4 * K],
                            in_=WT[kd * P:(kd + 1) * P, 3 * K:4 * K])
            ones_row = wp.tile([1, P], f32, tag="ones_row")
            nc.vector.memset(ones_row[:], 1.0)

            ut = [up.tile([P, NLOC], bf16, tag=f"ut{h}", name=f"ut{h}") for h in range(2)]
            csu_cols = [accp.tile([P, NB], f32, tag=f"csuc{h}", name=f"csuc{h}") for h in range(2)]
            # vtz_acc column 256 carries the colsum(V) partial (ones-column
            # trick). Two accumulators: A covers ibs [0, SPLIT), AllReduced
            # mid-phase-1 (absorbing the inter-core rendezvous while the PE is
            # still busy and the DMA fabric idle); B covers the rest + csu.
            SPLIT = 12
            vtz_acc = [
                [accp.tile([P, K + 1], f32, tag=f"vtz{ab}{h}", name=f"vtz{ab}{h}")
                 for h in range(2)] for ab in range(2)
            ]
            bin_a = dram.tile([2 * P, K + 1], f32)
            bout_a = dram.tile([2 * P, K + 1], f32)

            def t_pass(ib, wsrc):
                """T = relu(x @ Wt): 4 row-subtiles, one batched out-DMA."""
                for g in range(NSUB // 2):
                    otb = ob.tile([P, 2, K], f32, tag="ob")
                    for s2 in range(2):
                        s = g * 2 + s2
                        pt = ps.tile([P, K], f32, tag="work")
                        for kd in range(DT):
                            nc.tensor.matmul(
                                pt[:], xf[kd][:, ib * IB + s * P:ib * IB + (s + 1) * P],
                                wsrc[kd],
                                start=(kd == 0), stop=(kd == DT - 1),
                            )
                        nc.vector.tensor_relu(otb[:, s2, :], pt[:])
                    i0 = ib * IB + g * 2 * P
                    nc.sync.dma_start(
                        out=out[i0:i0 + 2 * P, K:2 * K].rearrange(
                            "(s p) c -> p s c", p=P),
                        in_=otb[:],
                    )

            wt_t = [wt[kd][:, 3 * K:4 * K] for kd in range(DT)]

            # ---- phase 1: projection + partial reductions ----
            for ib in range(NB):
                # U^T [k1, i] — stationary Wu^T, moving x^T; relu on ACT with
                # free-dim running sum (colsum_U partial) via accum_out.
                for h in range(2):
                    pu = ps.tile([P, IB], f32, tag="work")
                    for kd in range(DT):
                        nc.tensor.matmul(
                            pu[:], wt[kd][:, h * P:(h + 1) * P],
                            xf[kd][:, ib * IB:(ib + 1) * IB],
                            start=(kd == 0), stop=(kd == DT - 1),
                        )
                    nc.scalar.activation(
                        ut[h][:, ib * IB:(ib + 1) * IB], pu[:], RELU,
                        accum_out=csu_cols[h][:, ib:ib + 1],
                    )
                # V|Z in natural [i, j] layout per 128-row subtile; col 512 = 1.0
                vz_tiles = []
                for s in range(NSUB):
                    pvz = ps.tile([P, IB], f32, tag="work")
                    for kd in range(DT):
                        nc.tensor.matmul(
                            pvz[:], xf[kd][:, ib * IB + s * P:ib * IB + (s + 1) * P],
                            wt[kd][:, K:3 * K],
                            start=(kd == 0), stop=(kd == DT - 1),
                        )
                    vz = vzp.tile([P, 2 * K + 2], bf16, tag="vz")
                    nc.vector.tensor_relu(vz[:, 0:2 * K], pvz[:])
                    nc.vector.memset(vz[:, 2 * K:2 * K + 1], 1.0)
                    vz_tiles.append(vz)
                if ib < NB - TDEF:
                    t_pass(ib, wt_t)
                # VtZ|csV partial: contract i (partitions) over this block
                ab = 0 if ib < SPLIT else 1
                for h in range(2):
                    pz = ps.tile([P, K + 1], f32, tag="work")
                    for s in range(NSUB):
                        nc.tensor.matmul(
                            pz[:], vz_tiles[s][:, h * P:(h + 1) * P],
                            vz_tiles[s][:, K:2 * K + 1],
                            start=(s == 0), stop=(s == NSUB - 1),
                        )
                    if ib in (0, SPLIT):
                        nc.vector.tensor_copy(vtz_acc[ab][h][:], pz[:])
                    else:
                        nc.vector.tensor_add(vtz_acc[ab][h][:], vtz_acc[ab][h][:], pz[:])
                if ib == SPLIT - 1:
                    # launch AllReduce A: covers the bulk of VtZ while ibs
                    # [SPLIT, NB) still compute
                    for h in range(2):
                        nc.scalar.dma_start(
                            out=bin_a[h * P:(h + 1) * P, :], in_=vtz_acc[0][h][:]
                        )
                    nc.gpsimd.collective_compute(
                        "AllReduce", mybir.AluOpType.add,
                        replica_groups=[list(range(NCORES))],
                        ins=[bin_a.opt()], outs=[bout_a.opt()],
                    )

            # ---- phase 2: AllReduce [2*[k,k+1]] + [2*[k]] partials ----
            # Staging DMAs ride the scalar queue so they never wait behind the
            # sync queue's bulk traffic.
            csu = [accp.tile([P, 1], f32, tag=f"csu{h}", name=f"csu{h}") for h in range(2)]
            for h in range(2):
                nc.vector.reduce_sum(csu[h][:], csu_cols[h][:], axis=mybir.AxisListType.X)
            # Copy of the T-weight columns gated on a phase-1 output (gate==1.0
            # exactly, computed from csu): the deferred T matmuls read these
            # tiles, which truly pins them after phase 1 so they land inside
            # the AllReduce window instead of being hoisted into phase 1.
            gate = accp.tile([P, 1], f32, tag="gate")
            nc.vector.tensor_scalar(
                out=gate[:], in0=csu[0][:], scalar1=0.0, scalar2=1.0,
                op0=mybir.AluOpType.mult, op1=mybir.AluOpType.add,
            )
            wt2 = [wp.tile([P, K], bf16, tag=f"w2_{kd}", name=f"w2_{kd}") for kd in range(DT)]
            for kd in range(DT):
                nc.vector.tensor_scalar_mul(wt2[kd][:], wt[kd][:, 3 * K:4 * K], gate[:])
            bin_ = dram.tile([2 * P + 2, K + 1], f32)
            bout = dram.tile([2 * P + 2, K + 1], f32)
            for h in range(2):
                nc.scalar.dma_start(out=bin_[h * P:(h + 1) * P, :], in_=vtz_acc[1][h][:])
            for h in range(2):
                nc.scalar.dma_start(
                    out=bin_[2 * P + h, 0:P].rearrange("(p one) -> p one", one=1),
                    in_=csu[h][:],
                )
            nc.gpsimd.collective_compute(
                "AllReduce", mybir.AluOpType.add,
                replica_groups=[list(range(NCORES))],
                ins=[bin_.opt()], outs=[bout.opt()],
            )
            # ---- deferred T-pass: keeps PE busy/warm during the AllReduce ----
            wt2_t = [wt2[kd][:] for kd in range(DT)]
            for ib in range(NB - TDEF, NB):
                t_pass(ib, wt2_t)

            # ---- phase 3: D = 1/(csU.csV/n + eps); scale VtZ ----
            vtzf = [accp.tile([P, K + 1], f32, tag=f"vtzf{h}", name=f"vtzf{h}") for h in range(2)]
            vtzfb = accp.tile([P, K + 1], f32, tag="vtzfb")
            for h in range(2):
                nc.scalar.dma_start(out=vtzf[h][:], in_=bout_a[h * P:(h + 1) * P, :])
            for h in range(2):
                nc.scalar.dma_start(out=vtzfb[:], in_=bout[h * P:(h + 1) * P, :])
                nc.vector.tensor_add(vtzf[h][:], vtzf[h][:], vtzfb[:])
            csut = accp.tile([P, 2], f32, tag="csut")
            nc.scalar.dma_start(
                out=csut[:], in_=bout[2 * P:2 * P + 2, 0:P].rearrange("t p -> p t")
            )
            csvt = accp.tile([P, 2], f32, tag="csvt")
            for h in range(2):
                nc.vector.tensor_copy(csvt[:, h:h + 1], vtzf[h][:, K:K + 1])
            pdot = ps.tile([1, 1], f32, tag="work")
            for h in range(2):
                nc.tensor.matmul(
                    pdot[:], csut[:, h:h + 1], csvt[:, h:h + 1],
                    start=(h == 0), stop=(h == 1),
                )
            dsb = accp.tile([1, 1], f32, tag="dsb")
            nc.vector.tensor_scalar(
                out=dsb[:], in0=pdot[:], scalar1=1.0 / N_ROWS, scalar2=EPS,
                op0=mybir.AluOpType.mult, op1=mybir.AluOpType.add,
            )
            nc.vector.reciprocal(dsb[:], dsb[:])
            pb = ps.tile([P, 1], f32, tag="work")
            nc.tensor.matmul(pb[:], ones_row[:], dsb[:], start=True, stop=True)
            dbc = accp.tile([P, 1], f32, tag="dbc")
            nc.vector.tensor_copy(dbc[:], pb[:])
            vtzr = [accp.tile([P, K], bf16, tag=f"vtzr{h}", name=f"vtzr{h}") for h in range(2)]
            for h in range(2):
                nc.vector.tensor_scalar_mul(vtzr[h][:], vtzf[h][:, 0:K], dbc[:])

            # ---- phase 4: res = U @ (VtZ * D), batched row-natural writes ----
            # h-major over groups of 8 row-tiles: the moving operand stays
            # fixed for 8 consecutive matmuls and each PSUM start/stop pair is
            # spread apart, keeping the weight path warm.
            GG = 4
            for gb in range(NLOC // P // GG):
                prs = [ps.tile([P, K], f32, tag="work", name=f"pr{t}") for t in range(GG)]
                for h in range(2):
                    for t in range(GG):
                        i0 = (gb * GG + t) * P
                        nc.tensor.matmul(
                            prs[t][:], ut[h][:, i0:i0 + P], vtzr[h][:],
                            start=(h == 0), stop=(h == 1),
                        )
                for g2 in range(GG // 2):
                    orb = ob.tile([P, 2, K], f32, tag="ob")
                    for s2 in range(2):
                        t = g2 * 2 + s2
                        # split PSUM->SBUF copies across DVE and ACT: either
                        # alone is slower than the PE here
                        if s2 == 0:
                            nc.vector.tensor_copy(orb[:, s2, :], prs[t][:])
                        else:
                            nc.scalar.copy(orb[:, s2, :], prs[t][:])
                    i0 = (gb * GG + g2 * 2) * P
                    nc.sync.dma_start(
                        out=out[i0:i0 + 2 * P, 0:K].rearrange(
                            "(s p) c -> p s c", p=P),
                        in_=orb[:],
                    )

    nc.compile()
    return nc


def _get_nc(d_rows):
    if d_rows not in _built:
        _built[d_rows] = _build(d_rows)
    return _built[d_rows]


def _run(x, W, b, trace=False, trace_cores=None):
    from concourse.bass_utils import run_bass_kernel_spmd

    x = np.ascontiguousarray(x, dtype=np.float32)
    W = np.ascontiguousarray(W, dtype=np.float32)
    b = np.asarray(b, dtype=np.float32)
    if np.any(b):
        d_rows = 1152  # pad contraction: extra ones-row in x picks up b from W
        WT_full = np.zeros((d_rows, 4 * K), np.float32)
        WT_full[:D_IN] = W.T
        WT_full[D_IN] = b
    else:
        d_rows = D_IN
        WT_full = np.ascontiguousarray(W.T)
    WT_bf = WT_full.astype(BF16)
    nc = _get_nc(d_rows)
    in_maps = []
    for c in range(NCORES):
        xs = x[c * NLOC:(c + 1) * NLOC]
        if d_rows == D_IN:
            xTs = np.ascontiguousarray(xs.T.astype(BF16))
        else:
            xTs = np.zeros((d_rows, NLOC), BF16)
            xTs[:D_IN] = xs.T.astype(BF16)
            xTs[D_IN] = 1.0
        in_maps.append({"xT": xTs, "WT": WT_bf})
    res = run_bass_kernel_spmd(
        nc, in_maps, list(range(NCORES)),
        trace=trace, **({"trace_cores": trace_cores} if trace_cores else {}),
    )
    full = np.concatenate([res.results[c]["out"] for c in range(NCORES)], axis=0)
    return full, res


def kernel(x, W, b):
    full, _ = _run(x, W, b)
    return full



# revision 7
# speedup vs baseline: 1.1369x; 1.1369x over previous
"""Low-rank attention Trainium2 kernel (8 NeuronCores, SPMD), fp8 edition.

Math (reference):
    tmp = relu(x @ W.T + b); U,V,Z,T = split(tmp, 4, axis=1)
    norm = sum(U @ colsum(V)) / n + eps ;  D = 1/norm
    out = concat[(U @ (V.T @ Z)) * D, T]

Sharding: rows of x across 8 cores. Per-core partials (V.T@Z [k,k],
colsum(V), colsum(U)) are AllReduced on-device; each core then computes
its local U @ (VtZ) * D.

fp8 design (vs the 339us bf16 baseline):
- U, V, Z projections and V^T@Z run as fp8e4 DoubleRow matmuls (2 k-tiles
  per instruction, measured 216ns steady for moving-512 = true 2x bf16;
  LDWEIGHTS hides behind the previous matmul's streaming).
- x is quantized to fp8 on the HOST (x8 = e4m3(16x), 8MB/core, resident);
  on-device bf16->fp8 converts are not viable (only DVE writes fp8 fast).
- The T block stays bf16 (its error hits the output directly; fp8's ~2.5%
  elementwise would eat the whole 2e-2 budget). bf16 x streams through a
  rolling pool, one [1024, 512] block per T-pass block.
- Error budget (host sim vs fp64): total 4.3e-3 (res 7.9e-3 from fp8 via
  averaged quantization noise, T 2.0e-3 from bf16) vs 2e-2 gate.
- Scales: x8 = 16x, W8 = 64W, vz fp8 = 32*[V|Z]; U drains unscaled (bf16,
  ACT relu scale 1/1024 + csu accum); VtZ psum = 1024 V^T Z | 32 csV;
  constants folded into phase 3 (dsb 1/(32n), ones_row 1/1024).
- VZ fp8 drains on DVE (665ns each), U bf16 drains on ACT (439ns).
- Split AllReduce as baseline: VtZ partial for ibs [0,12) reduces
  mid-phase-1; remainder + csu at phase-1 end, hidden by deferred T-pass
  (TDEF blocks, gated copy of the T-weights pins them post-AllReduce).
- Phase 4 (U @ VtZ) kept bf16: it is floored by its own 8MB output DMA.
"""
import sys

sys.path.insert(0, "/opt/trn_rl_repo")
import numpy as np
import ml_dtypes

BF16 = ml_dtypes.bfloat16
E4 = ml_dtypes.float8_e4m3

NCORES = 8
N_ROWS, D_IN, K = 65536, 1024, 256
NLOC = N_ROWS // NCORES      # 8192 rows per core
P = 128
IB = 512                     # i-block width
NB = NLOC // IB              # 16 blocks
EPS = 1e-6
TDEF = 6                     # T-pass blocks deferred to overlap the AllReduce
SPLIT = 12                   # ibs [0, SPLIT) feed AllReduce A (mid-phase-1)
S_X, S_W, S_V = 16.0, 64.0, 32.0
X8CHUNKS = [(0, 512), (512, 512), (1024, 1024), (2048, 2048), (4096, 4096)]

_built = {}


def _build(d_rows):
    import concourse.bacc as bacc
    import concourse.mybir as mybir
    import concourse.tile as tile

    dt = mybir.dt
    f32, bf16, f8 = dt.float32, dt.bfloat16, dt.float8e4
    RELU = mybir.ActivationFunctionType.Relu
    DR = mybir.MatmulPerfMode.DoubleRow
    DT = d_rows // P
    KD2 = DT // 2            # DoubleRow kd-pairs
    KODD = DT % 2            # leftover plain-fp8 k-tile (bias-pad path)
    NSUB = IB // P
    SCL = 1.0 / (S_X * S_W)  # psum -> true pre-activation

    nc = bacc.Bacc("TRN2", target_bir_lowering=False, debug=False, num_devices=NCORES)
    x8d = nc.dram_tensor("x8", [d_rows, NLOC], f8, kind="ExternalInput")
    xbd = nc.dram_tensor("xb", [d_rows, NLOC], bf16, kind="ExternalInput")
    w8ud = nc.dram_tensor("w8u", [P, DT, K], f8, kind="ExternalInput")
    w8vzd = nc.dram_tensor("w8vz", [P, DT, 2 * K], f8, kind="ExternalInput")
    wttd = nc.dram_tensor("wtt", [P, DT, K], bf16, kind="ExternalInput")
    out = nc.dram_tensor("out", [NLOC, 2 * K], f32, kind="ExternalOutput")

    with tile.TileContext(nc) as tc:
        with (
            tc.tile_pool(name="wp", bufs=1) as wp,
            tc.tile_pool(name="xp", bufs=1) as xp,
            tc.tile_pool(name="xbp", bufs=8) as xbp,
            tc.tile_pool(name="up", bufs=1) as up,
            tc.tile_pool(name="vzp", bufs=4) as vzp,
            tc.tile_pool(name="ob", bufs=5) as ob,
            tc.tile_pool(name="acc", bufs=1) as accp,
            tc.tile_pool(name="ps", bufs=8, space="PSUM") as ps,
            tc.tile_pool(name="dram", bufs=1, space="DRAM") as dram,
        ):
            # Weights first (gpsimd/scalar queues), then resident x8 in
            # kd-interleaved chunks (sync queue, small first chunks so ib0
            # can start early). bf16 x streams per-block via xbp below.
            w8u = wp.tile([P, DT, K], f8, tag="w8u")
            nc.gpsimd.dma_start(out=w8u[:], in_=w8ud[:])
            w8vz = wp.tile([P, DT, 2 * K], f8, tag="w8vz")
            nc.scalar.dma_start(out=w8vz[:], in_=w8vzd[:])
            x8 = xp.tile([P, DT, NLOC], f8, tag="x8")
            for ci, (c0, cw) in enumerate(X8CHUNKS):
                nc.sync.dma_start(
                    out=x8[:, :, c0:c0 + cw],
                    in_=x8d[:, c0:c0 + cw].rearrange("(k p) c -> p k c", p=P),
                )
                if ci == 1:
                    wt = wp.tile([P, DT, K], bf16, tag="wt")
                    nc.scalar.dma_start(out=wt[:], in_=wttd[:])
            ones_row = wp.tile([1, P], f32, tag="ones_row")
            nc.vector.memset(ones_row[:], SCL)  # folds 1/1024 into D broadcast

            ut = [up.tile([P, NLOC], bf16, tag=f"ut{h}", name=f"ut{h}") for h in range(2)]
            csu_cols = [accp.tile([P, NB], f32, tag=f"csuc{h}", name=f"csuc{h}") for h in range(2)]
            # vtz_acc column 256 carries 32*colsum(V) (ones-column trick);
            # cols 0:256 hold 1024*V^T Z partials. Two accumulators: A covers
            # ibs [0, SPLIT), AllReduced mid-phase-1; B covers the rest + csu.
            vtz_acc = [
                [accp.tile([P, K + 1], f32, tag=f"vtz{ab}{h}", name=f"vtz{ab}{h}")
                 for h in range(2)] for ab in range(2)
            ]
            bin_a = dram.tile([2 * P, K + 1], f32)
            bout_a = dram.tile([2 * P, K + 1], f32)

            def load_xb(ib):
                xbt = xbp.tile([P, DT, IB], bf16, tag="xb", name=f"xb{ib}")
                nc.gpsimd.dma_start(
                    out=xbt[:],
                    in_=xbd[:, ib * IB:(ib + 1) * IB].rearrange("(k p) c -> p k c", p=P),
                )
                return xbt

            def t_pass(ib, xbt, wsrc):
                """T = relu(x @ Wt): 4 row-subtiles, one batched out-DMA."""
                for g in range(NSUB // 2):
                    otb = ob.tile([P, 2, K], f32, tag="ob")
                    for s2 in range(2):
                        s = g * 2 + s2
                        pt = ps.tile([P, K], f32, tag="work")
                        for kd in range(DT):
                            nc.tensor.matmul(
                                pt[:], xbt[:, kd, s * P:(s + 1) * P],
                                wsrc[:, kd, :],
                                start=(kd == 0), stop=(kd == DT - 1),
                            )
                        nc.vector.tensor_relu(otb[:, s2, :], pt[:])
                    i0 = ib * IB + g * 2 * P
                    nc.sync.dma_start(
                        out=out[i0:i0 + 2 * P, K:2 * K].rearrange(
                            "(s p) c -> p s c", p=P),
                        in_=otb[:],
                    )

            xbts = {}

            # ---- phase 1: fp8 projection + partial reductions ----
            for ib in range(NB):
                if ib < NB - TDEF:
                    xbts[ib] = load_xb(ib)
                # U^T [k1, i] — stationary Wu8 kd-pair, moving x8 kd-pair
                # (fp8 DoubleRow); relu+descale on ACT with free-dim running
                # sum (colsum_U partial) via accum_out.
                for h in range(2):
                    pu = ps.tile([P, IB], f32, tag="work")
                    for k2 in range(KD2):
                        nc.tensor.matmul(
                            pu[:], w8u[:, 2 * k2:2 * k2 + 2, h * P:(h + 1) * P],
                            x8[:, 2 * k2:2 * k2 + 2, ib * IB:(ib + 1) * IB],
                            start=(k2 == 0), stop=(k2 == KD2 - 1 and not KODD),
                            perf_mode=DR,
                        )
                    if KODD:
                        nc.tensor.matmul(
                            pu[:], w8u[:, DT - 1, h * P:(h + 1) * P],
                            x8[:, DT - 1, ib * IB:(ib + 1) * IB],
                            start=False, stop=True,
                        )
                    nc.scalar.activation(
                        ut[h][:, ib * IB:(ib + 1) * IB], pu[:], RELU, scale=SCL,
                        accum_out=csu_cols[h][:, ib:ib + 1],
                    )
                # V|Z natural [i, j] layout per 128-row subtile, drained to
                # fp8 (32*[V|Z]) on DVE; col 512 = 1.0 for the csV column.
                vz_tiles = []
                for sp in range(NSUB // 2):
                    vzt = vzp.tile([P, 2, 2 * K + 16], f8, tag="vz")
                    for s2 in range(2):
                        s = sp * 2 + s2
                        pvz = ps.tile([P, IB], f32, tag="work")
                        for k2 in range(KD2):
                            nc.tensor.matmul(
                                pvz[:],
                                x8[:, 2 * k2:2 * k2 + 2,
                                   ib * IB + s * P:ib * IB + (s + 1) * P],
                                w8vz[:, 2 * k2:2 * k2 + 2, :],
                                start=(k2 == 0), stop=(k2 == KD2 - 1 and not KODD),
                                perf_mode=DR,
                            )
                        if KODD:
                            nc.tensor.matmul(
                                pvz[:],
                                x8[:, DT - 1, ib * IB + s * P:ib * IB + (s + 1) * P],
                                w8vz[:, DT - 1, :],
                                start=False, stop=True,
                            )
                        nc.vector.tensor_scalar(
                            out=vzt[:, s2, 0:2 * K], in0=pvz[:],
                            scalar1=S_V * SCL, scalar2=0.0,
                            op0=mybir.AluOpType.mult, op1=mybir.AluOpType.max,
                        )
                    nc.vector.memset(vzt[:, :, 2 * K:2 * K + 1], 1.0)
                    vz_tiles.append(vzt)
                if ib < NB - TDEF:
                    t_pass(ib, xbts.pop(ib), wt)
                # VtZ|csV partial: contract i (partitions), s-pairs DoubleRow
                ab = 0 if ib < SPLIT else 1
                for h in range(2):
                    pz = ps.tile([P, K + 1], f32, tag="work")
                    for sp in range(NSUB // 2):
                        nc.tensor.matmul(
                            pz[:], vz_tiles[sp][:, :, h * P:(h + 1) * P],
                            vz_tiles[sp][:, :, K:2 * K + 1],
                            start=(sp == 0), stop=(sp == NSUB // 2 - 1),
                            perf_mode=DR,
                        )
                    if ib in (0, SPLIT):
                        nc.vector.tensor_copy(vtz_acc[ab][h][:], pz[:])
                    else:
                        nc.vector.tensor_add(vtz_acc[ab][h][:], vtz_acc[ab][h][:], pz[:])
                if ib == SPLIT - 1:
                    # launch AllReduce A: covers the bulk of VtZ while ibs
                    # [SPLIT, NB) still compute
                    for h in range(2):
                        nc.scalar.dma_start(
                            out=bin_a[h * P:(h + 1) * P, :], in_=vtz_acc[0][h][:]
                        )
                    nc.gpsimd.collective_compute(
                        "AllReduce", mybir.AluOpType.add,
                        replica_groups=[list(range(NCORES))],
                        ins=[bin_a.opt()], outs=[bout_a.opt()],
                    )
                    for dib in range(NB - TDEF, NB):
                        xbts[dib] = load_xb(dib)

            # ---- phase 2: AllReduce [2*[k,k+1]] + [2*[k]] partials ----
            # Staging DMAs ride the scalar queue so they never wait behind the
            # sync queue's bulk traffic.
            csu = [accp.tile([P, 1], f32, tag=f"csu{h}", name=f"csu{h}") for h in range(2)]
            for h in range(2):
                nc.vector.reduce_sum(csu[h][:], csu_cols[h][:], axis=mybir.AxisListType.X)
            # Copy of the T-weight columns gated on a phase-1 output (gate==1.0
            # exactly, computed from csu): the deferred T matmuls read these
            # tiles, which truly pins them after phase 1 so they land inside
            # the AllReduce window instead of being hoisted into phase 1.
            gate = accp.tile([P, 1], f32, tag="gate")
            nc.vector.tensor_scalar(
                out=gate[:], in0=csu[0][:], scalar1=0.0, scalar2=1.0,
                op0=mybir.AluOpType.mult, op1=mybir.AluOpType.add,
            )
            wt2 = wp.tile([P, DT, K], bf16, tag="wt2")
            nc.vector.tensor_scalar_mul(wt2[:], wt[:], gate[:])
            bin_ = dram.tile([2 * P + 2, K + 1], f32)
            bout = dram.tile([2 * P + 2, K + 1], f32)
            for h in range(2):
                nc.scalar.dma_start(out=bin_[h * P:(h + 1) * P, :], in_=vtz_acc[1][h][:])
            for h in range(2):
                nc.scalar.dma_start(
                    out=bin_[2 * P + h, 0:P].rearrange("(p one) -> p one", one=1),
                    in_=csu[h][:],
                )
            nc.gpsimd.collective_compute(
                "AllReduce", mybir.AluOpType.add,
                replica_groups=[list(range(NCORES))],
                ins=[bin_.opt()], outs=[bout.opt()],
            )
            # ---- deferred T-pass: keeps PE busy/warm during the AllReduce ----
            for ib in range(NB - TDEF, NB):
                t_pass(ib, xbts.pop(ib), wt2)

            # ---- phase 3: D = 1/(csU.csV/n + eps); scale VtZ ----
            vtzf = [accp.tile([P, K + 1], f32, tag=f"vtzf{h}", name=f"vtzf{h}") for h in range(2)]
            vtzfb = accp.tile([P, K + 1], f32, tag="vtzfb")
            for h in range(2):
                nc.scalar.dma_start(out=vtzf[h][:], in_=bout_a[h * P:(h + 1) * P, :])
            for h in range(2):
                nc.scalar.dma_start(out=vtzfb[:], in_=bout[h * P:(h + 1) * P, :])
                nc.vector.tensor_add(vtzf[h][:], vtzf[h][:], vtzfb[:])
            csut = accp.tile([P, 2], f32, tag="csut")
            nc.scalar.dma_start(
                out=csut[:], in_=bout[2 * P:2 * P + 2, 0:P].rearrange("t p -> p t")
            )
            csvt = accp.tile([P, 2], f32, tag="csvt")
            for h in range(2):
                nc.vector.tensor_copy(csvt[:, h:h + 1], vtzf[h][:, K:K + 1])
            pdot = ps.tile([1, 1], f32, tag="work")
            for h in range(2):
                nc.tensor.matmul(
                    pdot[:], csut[:, h:h + 1], csvt[:, h:h + 1],
                    start=(h == 0), stop=(h == 1),
                )
            dsb = accp.tile([1, 1], f32, tag="dsb")
            nc.vector.tensor_scalar(
                out=dsb[:], in0=pdot[:], scalar1=1.0 / (S_V * N_ROWS), scalar2=EPS,
                op0=mybir.AluOpType.mult, op1=mybir.AluOpType.add,
            )
            nc.vector.reciprocal(dsb[:], dsb[:])
            pb = ps.tile([P, 1], f32, tag="work")
            nc.tensor.matmul(pb[:], ones_row[:], dsb[:], start=True, stop=True)
            dbc = accp.tile([P, 1], f32, tag="dbc")
            nc.vector.tensor_copy(dbc[:], pb[:])
            vtzr = [accp.tile([P, K], bf16, tag=f"vtzr{h}", name=f"vtzr{h}") for h in range(2)]
            for h in range(2):
                nc.vector.tensor_scalar_mul(vtzr[h][:], vtzf[h][:, 0:K], dbc[:])

            # ---- phase 4: res = U @ (VtZ * D), batched row-natural writes ----
            # h-major over groups of 4 PSUM tiles: the moving operand stays
            # fixed for the group and each start/stop pair is spread apart,
            # keeping the weight path warm.
            GG = 4
            for gb in range(NLOC // P // GG):
                prs = [ps.tile([P, K], f32, tag="work", name=f"pr{t}") for t in range(GG)]
                for h in range(2):
                    for t in range(GG):
                        i0 = (gb * GG + t) * P
                        nc.tensor.matmul(
                            prs[t][:], ut[h][:, i0:i0 + P], vtzr[h][:],
                            start=(h == 0), stop=(h == 1),
                        )
                for g2 in range(GG // 2):
                    orb = ob.tile([P, 2, K], f32, tag="ob")
                    for s2 in range(2):
                        t = g2 * 2 + s2
                        # split PSUM->SBUF copies across DVE and ACT: either
                        # alone is slower than the PE here
                        if s2 == 0:
                            nc.vector.tensor_copy(orb[:, s2, :], prs[t][:])
                        else:
                            nc.scalar.copy(orb[:, s2, :], prs[t][:])
                    i0 = (gb * GG + g2 * 2) * P
                    nc.sync.dma_start(
                        out=out[i0:i0 + 2 * P, 0:K].rearrange(
                            "(s p) c -> p s c", p=P),
                        in_=orb[:],
                    )

    nc.compile()
    return nc


def _get_nc(d_rows):
    if d_rows not in _built:
        _built[d_rows] = _build(d_rows)
    return _built[d_rows]


def _q8(a, s):
    return np.clip(a * s, -240.0, 240.0).astype(E4)


def _run(x, W, b, trace=False, trace_cores=None):
    from concourse.bass_utils import run_bass_kernel_spmd

    x = np.ascontiguousarray(x, dtype=np.float32)
    W = np.ascontiguousarray(W, dtype=np.float32)
    b = np.asarray(b, dtype=np.float32)
    if np.any(b):
        d_rows = 1152  # pad contraction: extra ones-row in x picks up b from W
        WT_full = np.zeros((d_rows, 4 * K), np.float32)
        WT_full[:D_IN] = W.T
        WT_full[D_IN] = b
    else:
        d_rows = D_IN
        WT_full = np.ascontiguousarray(W.T)
    DT = d_rows // P
    w8u = np.ascontiguousarray(
        _q8(WT_full[:, 0:K], S_W).reshape(DT, P, K).transpose(1, 0, 2))
    w8vz = np.ascontiguousarray(
        _q8(WT_full[:, K:3 * K], S_W).reshape(DT, P, 2 * K).transpose(1, 0, 2))
    wtt = np.ascontiguousarray(
        WT_full[:, 3 * K:].astype(BF16).reshape(DT, P, K).transpose(1, 0, 2))
    nc = _get_nc(d_rows)
    in_maps = []
    for c in range(NCORES):
        xs = x[c * NLOC:(c + 1) * NLOC]
        if d_rows == D_IN:
            xTs = np.ascontiguousarray(xs.T)
        else:
            xTs = np.zeros((d_rows, NLOC), np.float32)
            xTs[:D_IN] = xs.T
            xTs[D_IN] = 1.0
        xb_bf = xTs.astype(BF16)
        x8_ = _q8(xb_bf.astype(np.float32), S_X)
        in_maps.append({"x8": x8_, "xb": xb_bf, "w8u": w8u, "w8vz": w8vz, "wtt": wtt})
    res = run_bass_kernel_spmd(
        nc, in_maps, list(range(NCORES)),
        trace=trace, **({"trace_cores": trace_cores} if trace_cores else {}),
    )
    full = np.concatenate([res.results[c]["out"] for c in range(NCORES)], axis=0)
    return full, res


def kernel(x, W, b):
    full, _ = _run(x, W, b)
    return full


# revision 8
# speedup vs baseline: 1.1449x; 1.0070x over previous
"""Low-rank attention Trainium2 kernel (8 NeuronCores, SPMD), fp8 edition.

Math (reference):
    tmp = relu(x @ W.T + b); U,V,Z,T = split(tmp, 4, axis=1)
    norm = sum(U @ colsum(V)) / n + eps ;  D = 1/norm
    out = concat[(U @ (V.T @ Z)) * D, T]

Sharding: rows of x across 8 cores. Per-core partials (V.T@Z [k,k],
colsum(V), colsum(U)) are AllReduced on-device; each core then computes
its local U @ (VtZ) * D.

fp8 design (vs the 339us bf16 baseline):
- U, V, Z projections and V^T@Z run as fp8e4 DoubleRow matmuls (2 k-tiles
  per instruction, measured 216ns steady for moving-512 = true 2x bf16;
  LDWEIGHTS hides behind the previous matmul's streaming).
- x is quantized to fp8 on the HOST (x8 = e4m3(16x), 8MB/core, resident);
  on-device bf16->fp8 converts are not viable (only DVE writes fp8 fast).
- The T block stays bf16 (its error hits the output directly; fp8's ~2.5%
  elementwise would eat the whole 2e-2 budget). bf16 x streams through a
  rolling pool, one [1024, 512] block per T-pass block.
- Scales: x8 = 16x, W8 = 64W, vz fp8 = 32*[V|Z]; U drains unscaled (bf16,
  ACT relu scale 1/1024 + csu accum); VtZ psum = 1024 V^T Z | 32 csV.

Collective hiding (the bf16 baseline exposed ~41us of AllReduce):
- Phase 1a: V|Z + V^T@Z for ALL i-blocks first; V^T@Z accumulates across
  blocks in two PSUM chains (no per-block DVE adds). AllReduce A (the
  whole [k,k+1] x 2 payload) launches at ~1/3 of the kernel.
- Phase 1b: all U-passes; then AllReduce C (csu only, 1KB).
- T-passes follow (last TDEF read a csu-gated copy of the T-weights,
  pinning them after C's launch); phase-4 matmuls need only A's result
  (vtzr = V^T Z unscaled); the data-dependent D = 1/norm is applied at
  the phase-4 PSUM drains as a per-partition AP scale, so C's latency
  hides under the T-pass + phase-4 matmuls.
- DMA rings: x8 + T-out on sync, weights + staging + res-out on scalar,
  xb blocks + collectives on gpsimd.
"""
import sys

sys.path.insert(0, "/opt/trn_rl_repo")
import numpy as np
import ml_dtypes

BF16 = ml_dtypes.bfloat16
E4 = ml_dtypes.float8_e4m3

NCORES = 8
N_ROWS, D_IN, K = 65536, 1024, 256
NLOC = N_ROWS // NCORES      # 8192 rows per core
P = 128
IB = 512                     # i-block width
NB = NLOC // IB              # 16 blocks
EPS = 1e-6
TDEF = 6                     # T-pass blocks deferred behind AllReduce C
S_X, S_W, S_V = 16.0, 64.0, 32.0
X8CHUNKS = [(0, 512), (512, 512), (1024, 1024), (2048, 2048), (4096, 4096)]

_built = {}


def _build(d_rows):
    import concourse.bacc as bacc
    import concourse.mybir as mybir
    import concourse.tile as tile

    dt = mybir.dt
    f32, bf16, f8 = dt.float32, dt.bfloat16, dt.float8e4
    RELU = mybir.ActivationFunctionType.Relu
    DR = mybir.MatmulPerfMode.DoubleRow
    DT = d_rows // P
    KD2 = DT // 2            # DoubleRow kd-pairs
    KODD = DT % 2            # leftover plain-fp8 k-tile (bias-pad path)
    NSUB = IB // P
    SCL = 1.0 / (S_X * S_W)  # psum -> true pre-activation

    nc = bacc.Bacc("TRN2", target_bir_lowering=False, debug=False, num_devices=NCORES)
    x8d = nc.dram_tensor("x8", [d_rows, NLOC], f8, kind="ExternalInput")
    xbd = nc.dram_tensor("xb", [d_rows, NLOC], bf16, kind="ExternalInput")
    w8ud = nc.dram_tensor("w8u", [P, DT, K], f8, kind="ExternalInput")
    w8vzd = nc.dram_tensor("w8vz", [P, DT, 2 * K], f8, kind="ExternalInput")
    wttd = nc.dram_tensor("wtt", [P, DT, K], bf16, kind="ExternalInput")
    out = nc.dram_tensor("out", [NLOC, 2 * K], f32, kind="ExternalOutput")

    with tile.TileContext(nc) as tc:
        with (
            tc.tile_pool(name="wp", bufs=1) as wp,
            tc.tile_pool(name="xp", bufs=1) as xp,
            tc.tile_pool(name="xbp", bufs=8) as xbp,
            tc.tile_pool(name="up", bufs=1) as up,
            tc.tile_pool(name="vzp", bufs=4) as vzp,
            tc.tile_pool(name="ob", bufs=5) as ob,
            tc.tile_pool(name="acc", bufs=1) as accp,
            tc.tile_pool(name="ps", bufs=6, space="PSUM") as ps,
            tc.tile_pool(name="ps2", bufs=1, space="PSUM") as ps2,
            tc.tile_pool(name="dram", bufs=1, space="DRAM") as dram,
        ):
            # Weights (gpsimd/scalar rings), then resident x8 in per-kd
            # column chunks (sync ring, small first chunks so ib0 starts
            # early). bf16 x streams per-block via xbp below.
            w8u = wp.tile([P, DT, K], f8, tag="w8u")
            nc.gpsimd.dma_start(out=w8u[:], in_=w8ud[:])
            w8vz = wp.tile([P, DT, 2 * K], f8, tag="w8vz")
            nc.scalar.dma_start(out=w8vz[:], in_=w8vzd[:])
            x8 = xp.tile([P, DT, NLOC], f8, tag="x8")
            for ci, (c0, cw) in enumerate(X8CHUNKS):
                for kd in range(DT):
                    nc.sync.dma_start(
                        out=x8[:, kd, c0:c0 + cw],
                        in_=x8d[kd * P:(kd + 1) * P, c0:c0 + cw],
                    )
                if ci == 1:
                    wt = wp.tile([P, DT, K], bf16, tag="wt")
                    nc.scalar.dma_start(out=wt[:], in_=wttd[:])
            ones_row = wp.tile([1, P], f32, tag="ones_row")
            nc.vector.memset(ones_row[:], 1.0)

            ut = [up.tile([P, NLOC], bf16, tag=f"ut{h}", name=f"ut{h}") for h in range(2)]
            csu_cols = [accp.tile([P, NB], f32, tag=f"csuc{h}", name=f"csuc{h}") for h in range(2)]

            def load_xb(ib):
                xbt = xbp.tile([P, DT, IB], bf16, tag="xb", name=f"xb{ib}")
                nc.gpsimd.dma_start(
                    out=xbt[:],
                    in_=xbd[:, ib * IB:(ib + 1) * IB].rearrange("(k p) c -> p k c", p=P),
                )
                return xbt

            def t_pass(ib, xbt, wsrc):
                """T = relu(x @ Wt): 4 row-subtiles, one batched out-DMA."""
                for g in range(NSUB // 2):
                    otb = ob.tile([P, 2, K], f32, tag="ob")
                    for s2 in range(2):
                        s = g * 2 + s2
                        pt = ps.tile([P, K], f32, tag="work")
                        for kd in range(DT):
                            nc.tensor.matmul(
                                pt[:], xbt[:, kd, s * P:(s + 1) * P],
                                wsrc[:, kd, :],
                                start=(kd == 0), stop=(kd == DT - 1),
                            )
                        nc.vector.tensor_relu(otb[:, s2, :], pt[:])
                    i0 = ib * IB + g * 2 * P
                    nc.sync.dma_start(
                        out=out[i0:i0 + 2 * P, K:2 * K].rearrange(
                            "(s p) c -> p s c", p=P),
                        in_=otb[:],
                    )

            # ---- phase 1a: V|Z fp8 projection + V^T@Z PSUM chains ----
            # vz col 512 = 1.0 rides the V^T@Z matmul to produce 32*csV in
            # column 256 of the [k, k+1] chain.
            pzh = [ps2.tile([P, K + 1], f32, tag=f"pz{h}", name=f"pz{h}") for h in range(2)]
            for ib in range(NB):
                vz_tiles = []
                for sp in range(NSUB // 2):
                    vzt = vzp.tile([P, 2, 2 * K + 16], f8, tag="vz")
                    for s2 in range(2):
                        s = sp * 2 + s2
                        pvz = ps.tile([P, IB], f32, tag="work")
                        for k2 in range(KD2):
                            nc.tensor.matmul(
                                pvz[:],
                                x8[:, 2 * k2:2 * k2 + 2,
                                   ib * IB + s * P:ib * IB + (s + 1) * P],
                                w8vz[:, 2 * k2:2 * k2 + 2, :],
                                start=(k2 == 0), stop=(k2 == KD2 - 1 and not KODD),
                                perf_mode=DR,
                            )
                        if KODD:
                            nc.tensor.matmul(
                                pvz[:],
                                x8[:, DT - 1, ib * IB + s * P:ib * IB + (s + 1) * P],
                                w8vz[:, DT - 1, :],
                                start=False, stop=True,
                            )
                        nc.vector.tensor_scalar(
                            out=vzt[:, s2, 0:2 * K], in0=pvz[:],
                            scalar1=S_V * SCL, scalar2=0.0,
                            op0=mybir.AluOpType.mult, op1=mybir.AluOpType.max,
                        )
                    nc.vector.memset(vzt[:, :, 2 * K:2 * K + 1], 1.0)
                    vz_tiles.append(vzt)
                for h in range(2):
                    for sp in range(NSUB // 2):
                        nc.tensor.matmul(
                            pzh[h][:], vz_tiles[sp][:, :, h * P:(h + 1) * P],
                            vz_tiles[sp][:, :, K:2 * K + 1],
                            start=(ib == 0 and sp == 0),
                            stop=(ib == NB - 1 and sp == NSUB // 2 - 1),
                            perf_mode=DR,
                        )

            # ---- AllReduce A: the full V^T@Z | csV payload ----
            bin_a = dram.tile([2 * P, K + 1], f32)
            bout_a = dram.tile([2 * P, K + 1], f32)
            vtzs = [accp.tile([P, K + 1], f32, tag=f"vtzs{h}", name=f"vtzs{h}") for h in range(2)]
            for h in range(2):
                nc.vector.tensor_copy(vtzs[h][:], pzh[h][:])
                nc.scalar.dma_start(out=bin_a[h * P:(h + 1) * P, :], in_=vtzs[h][:])
            nc.gpsimd.collective_compute(
                "AllReduce", mybir.AluOpType.add,
                replica_groups=[list(range(NCORES))],
                ins=[bin_a.opt()], outs=[bout_a.opt()],
            )

            # ---- phase 1b: all U-passes (fp8 DR), then AllReduce C (csu) ----
            for ib in range(NB):
                for h in range(2):
                    pu = ps.tile([P, IB], f32, tag="work")
                    for k2 in range(KD2):
                        nc.tensor.matmul(
                            pu[:], w8u[:, 2 * k2:2 * k2 + 2, h * P:(h + 1) * P],
                            x8[:, 2 * k2:2 * k2 + 2, ib * IB:(ib + 1) * IB],
                            start=(k2 == 0), stop=(k2 == KD2 - 1 and not KODD),
                            perf_mode=DR,
                        )
                    if KODD:
                        nc.tensor.matmul(
                            pu[:], w8u[:, DT - 1, h * P:(h + 1) * P],
                            x8[:, DT - 1, ib * IB:(ib + 1) * IB],
                            start=False, stop=True,
                        )
                    nc.scalar.activation(
                        ut[h][:, ib * IB:(ib + 1) * IB], pu[:], RELU, scale=SCL,
                        accum_out=csu_cols[h][:, ib:ib + 1],
                    )

            csu = [accp.tile([P, 1], f32, tag=f"csu{h}", name=f"csu{h}") for h in range(2)]
            for h in range(2):
                nc.vector.reduce_sum(csu[h][:], csu_cols[h][:], axis=mybir.AxisListType.X)
            bin_c = dram.tile([2, P], f32)
            bout_c = dram.tile([2, P], f32)
            for h in range(2):
                nc.scalar.dma_start(
                    out=bin_c[h, 0:P].rearrange("(p one) -> p one", one=1),
                    in_=csu[h][:],
                )
            nc.gpsimd.collective_compute(
                "AllReduce", mybir.AluOpType.add,
                replica_groups=[list(range(NCORES))],
                ins=[bin_c.opt()], outs=[bout_c.opt()],
            )
            # Gated copy of the T-weights (gate==1.0 exactly, from csu): the
            # deferred T matmuls read wt2, pinning them after C's launch so
            # they fill C's flight time instead of being hoisted earlier.
            gate = accp.tile([P, 1], f32, tag="gate")
            nc.vector.tensor_scalar(
                out=gate[:], in0=csu[0][:], scalar1=0.0, scalar2=1.0,
                op0=mybir.AluOpType.mult, op1=mybir.AluOpType.add,
            )
            wt2 = wp.tile([P, DT, K], bf16, tag="wt2")
            nc.vector.tensor_scalar_mul(wt2[:], wt[:], gate[:])

            # vtzr = V^T Z (unscaled, bf16) — needs only AllReduce A.
            vtzf = [accp.tile([P, K + 1], f32, tag=f"vtzf{h}", name=f"vtzf{h}") for h in range(2)]
            for h in range(2):
                nc.scalar.dma_start(out=vtzf[h][:], in_=bout_a[h * P:(h + 1) * P, :])
            vtzr = [accp.tile([P, K], bf16, tag=f"vtzr{h}", name=f"vtzr{h}") for h in range(2)]
            for h in range(2):
                nc.vector.tensor_scalar_mul(vtzr[h][:], vtzf[h][:, 0:K], SCL)

            # ---- T-passes (xb streamed per block; last TDEF read wt2) ----
            for ib in range(NB):
                t_pass(ib, load_xb(ib), wt if ib < NB - TDEF else wt2)

            # ---- phase 3: D = 1/(csU.csV/n + eps) as a drain-time scale ----
            csut = accp.tile([P, 2], f32, tag="csut")
            nc.scalar.dma_start(out=csut[:], in_=bout_c.rearrange("t p -> p t"))
            csvt = accp.tile([P, 2], f32, tag="csvt")
            for h in range(2):
                nc.vector.tensor_copy(csvt[:, h:h + 1], vtzf[h][:, K:K + 1])
            pdot = ps.tile([1, 1], f32, tag="work")
            for h in range(2):
                nc.tensor.matmul(
                    pdot[:], csut[:, h:h + 1], csvt[:, h:h + 1],
                    start=(h == 0), stop=(h == 1),
                )
            dsb = accp.tile([1, 1], f32, tag="dsb")
            nc.vector.tensor_scalar(
                out=dsb[:], in0=pdot[:], scalar1=1.0 / (S_V * N_ROWS), scalar2=EPS,
                op0=mybir.AluOpType.mult, op1=mybir.AluOpType.add,
            )
            nc.vector.reciprocal(dsb[:], dsb[:])
            pb = ps.tile([P, 1], f32, tag="work")
            nc.tensor.matmul(pb[:], ones_row[:], dsb[:], start=True, stop=True)
            dbc = accp.tile([P, 1], f32, tag="dbc")
            nc.vector.tensor_copy(dbc[:], pb[:])

            # ---- phase 4: res = (U @ VtZ) * D, batched row-natural writes ----
            # h-major over groups of 4 PSUM tiles: the moving operand stays
            # fixed for the group and each start/stop pair is spread apart,
            # keeping the weight path warm. D lands at the drains (AP scale).
            GG = 4
            for gb in range(NLOC // P // GG):
                prs = [ps.tile([P, K], f32, tag="work", name=f"pr{t}") for t in range(GG)]
                for h in range(2):
                    for t in range(GG):
                        i0 = (gb * GG + t) * P
                        nc.tensor.matmul(
                            prs[t][:], ut[h][:, i0:i0 + P], vtzr[h][:],
                            start=(h == 0), stop=(h == 1),
                        )
                for g2 in range(GG // 2):
                    orb = ob.tile([P, 2, K], f32, tag="ob")
                    for s2 in range(2):
                        t = g2 * 2 + s2
                        # split PSUM->SBUF scaled copies across DVE and ACT
                        if s2 == 0:
                            nc.vector.tensor_scalar_mul(orb[:, s2, :], prs[t][:], dbc[:])
                        else:
                            nc.scalar.mul(orb[:, s2, :], prs[t][:], dbc[:])
                    i0 = (gb * GG + g2 * 2) * P
                    nc.scalar.dma_start(
                        out=out[i0:i0 + 2 * P, 0:K].rearrange(
                            "(s p) c -> p s c", p=P),
                        in_=orb[:],
                    )

    nc.compile()
    return nc


def _get_nc(d_rows):
    if d_rows not in _built:
        _built[d_rows] = _build(d_rows)
    return _built[d_rows]


def _q8(a, s):
    return np.clip(a * s, -240.0, 240.0).astype(E4)


def _run(x, W, b, trace=False, trace_cores=None):
    from concourse.bass_utils import run_bass_kernel_spmd

    x = np.ascontiguousarray(x, dtype=np.float32)
    W = np.ascontiguousarray(W, dtype=np.float32)
    b = np.asarray(b, dtype=np.float32)
    if np.any(b):
        d_rows = 1152  # pad contraction: extra ones-row in x picks up b from W
        WT_full = np.zeros((d_rows, 4 * K), np.float32)
        WT_full[:D_IN] = W.T
        WT_full[D_IN] = b
    else:
        d_rows = D_IN
        WT_full = np.ascontiguousarray(W.T)
    DT = d_rows // P
    w8u = np.ascontiguousarray(
        _q8(WT_full[:, 0:K], S_W).reshape(DT, P, K).transpose(1, 0, 2))
    w8vz = np.ascontiguousarray(
        _q8(WT_full[:, K:3 * K], S_W).reshape(DT, P, 2 * K).transpose(1, 0, 2))
    wtt = np.ascontiguousarray(
        WT_full[:, 3 * K:].astype(BF16).reshape(DT, P, K).transpose(1, 0, 2))
    nc = _get_nc(d_rows)
    in_maps = []
    for c in range(NCORES):
        xs = x[c * NLOC:(c + 1) * NLOC]
        if d_rows == D_IN:
            xTs = np.ascontiguousarray(xs.T)
        else:
            xTs = np.zeros((d_rows, NLOC), np.float32)
            xTs[:D_IN] = xs.T
            xTs[D_IN] = 1.0
        xb_bf = xTs.astype(BF16)
        x8_ = _q8(xb_bf.astype(np.float32), S_X)
        in_maps.append({"x8": x8_, "xb": xb_bf, "w8u": w8u, "w8vz": w8vz, "wtt": wtt})
    res = run_bass_kernel_spmd(
        nc, in_maps, list(range(NCORES)),
        trace=trace, **({"trace_cores": trace_cores} if trace_cores else {}),
    )
    full = np.concatenate([res.results[c]["out"] for c in range(NCORES)], axis=0)
    return full, res


def kernel(x, W, b):
    full, _ = _run(x, W, b)
    return full


# revision 10
# speedup vs baseline: 1.1628x; 1.0156x over previous
"""Low-rank attention Trainium2 kernel (8 NeuronCores, SPMD), fp8 edition.

Math (reference):
    tmp = relu(x @ W.T + b); U,V,Z,T = split(tmp, 4, axis=1)
    norm = sum(U @ colsum(V)) / n + eps ;  D = 1/norm
    out = concat[(U @ (V.T @ Z)) * D, T]

Sharding: rows of x across 8 cores. Per-core partials (V.T@Z [k,k],
colsum(V), colsum(U)) are AllReduced on-device; each core then computes
its local U @ (VtZ) * D.

fp8 design (vs the 339us bf16 baseline):
- U, V, Z projections and V^T@Z run as fp8e4 DoubleRow matmuls (2 k-tiles
  per instruction, measured 216ns steady for moving-512 = true 2x bf16;
  LDWEIGHTS hides behind the previous matmul's streaming).
- x is quantized to fp8 on the HOST (x8 = e4m3(16x), 8MB/core, resident);
  on-device bf16->fp8 converts are not viable (only DVE writes fp8 fast).
- The T block stays bf16 (its error hits the output directly; fp8's ~2.5%
  elementwise would eat the whole 2e-2 budget). bf16 x streams through a
  rolling pool, one [1024, 512] block per T-pass block.
- Scales: x8 = 16x, W8 = 64W, vz fp8 = 32*[V|Z]; U drains unscaled (bf16,
  ACT relu scale 1/1024 + csu accum); VtZ psum = 1024 V^T Z | 32 csV.

Collective hiding (the bf16 baseline exposed ~41us of AllReduce):
- Phase 1a: V|Z + V^T@Z for ALL i-blocks first; V^T@Z accumulates across
  blocks in two PSUM chains (no per-block DVE adds). AllReduce A (the
  whole [k,k+1] x 2 payload) launches at ~1/3 of the kernel.
- Phase 1b: all U-passes; then AllReduce C (csu only, 1KB).
- T-passes follow (last TDEF read a csu-gated copy of the T-weights,
  pinning them after C's launch); phase-4 matmuls need only A's result
  (vtzr = V^T Z unscaled); the data-dependent D = 1/norm is applied at
  the phase-4 PSUM drains as a per-partition AP scale, so C's latency
  hides under the T-pass + phase-4 matmuls.
- DMA rings: x8 + T-out on sync, weights + staging + res-out on scalar,
  xb blocks + collectives on gpsimd.
"""
import sys

sys.path.insert(0, "/opt/trn_rl_repo")
import numpy as np
import ml_dtypes

BF16 = ml_dtypes.bfloat16
E4 = ml_dtypes.float8_e4m3

NCORES = 8
N_ROWS, D_IN, K = 65536, 1024, 256
NLOC = N_ROWS // NCORES      # 8192 rows per core
P = 128
IB = 512                     # i-block width
NB = NLOC // IB              # 16 blocks
EPS = 1e-6
TDEF = 6                     # T-pass blocks deferred behind AllReduce C
S_X, S_W, S_V = 16.0, 64.0, 32.0
X8CHUNKS = [(0, 512), (512, 512), (1024, 1024), (2048, 2048), (4096, 4096)]

_built = {}


def _build(d_rows):
    import concourse.bacc as bacc
    import concourse.mybir as mybir
    import concourse.tile as tile

    dt = mybir.dt
    f32, bf16, f8 = dt.float32, dt.bfloat16, dt.float8e4
    RELU = mybir.ActivationFunctionType.Relu
    DR = mybir.MatmulPerfMode.DoubleRow
    DT = d_rows // P
    KD2 = DT // 2            # DoubleRow kd-pairs
    KODD = DT % 2            # leftover plain-fp8 k-tile (bias-pad path)
    NSUB = IB // P
    SCL = 1.0 / (S_X * S_W)  # psum -> true pre-activation

    nc = bacc.Bacc("TRN2", target_bir_lowering=False, debug=False, num_devices=NCORES)
    x8d = nc.dram_tensor("x8", [d_rows, NLOC], f8, kind="ExternalInput")
    xbd = nc.dram_tensor("xb", [d_rows, NLOC], bf16, kind="ExternalInput")
    w8ud = nc.dram_tensor("w8u", [P, DT, K], f8, kind="ExternalInput")
    w8vzd = nc.dram_tensor("w8vz", [P, DT, 2 * K], f8, kind="ExternalInput")
    wttd = nc.dram_tensor("wtt", [P, DT, K], bf16, kind="ExternalInput")
    out = nc.dram_tensor("out", [NLOC, 2 * K], f32, kind="ExternalOutput")

    with tile.TileContext(nc) as tc:
        with (
            tc.tile_pool(name="wp", bufs=1) as wp,
            tc.tile_pool(name="xp", bufs=1) as xp,
            tc.tile_pool(name="xbp", bufs=8) as xbp,
            tc.tile_pool(name="up", bufs=1) as up,
            tc.tile_pool(name="vzp", bufs=4) as vzp,
            tc.tile_pool(name="ob", bufs=5) as ob,
            tc.tile_pool(name="acc", bufs=1) as accp,
            tc.tile_pool(name="ps", bufs=6, space="PSUM") as ps,
            tc.tile_pool(name="ps2", bufs=1, space="PSUM") as ps2,
            tc.tile_pool(name="dram", bufs=1, space="DRAM") as dram,
        ):
            # Weights (gpsimd/scalar rings), then resident x8 in per-kd
            # column chunks (sync ring, small first chunks so ib0 starts
            # early). bf16 x streams per-block via xbp below.
            w8u = wp.tile([P, DT, K], f8, tag="w8u")
            nc.gpsimd.dma_start(out=w8u[:], in_=w8ud[:])
            w8vz = wp.tile([P, DT, 2 * K], f8, tag="w8vz")
            nc.scalar.dma_start(out=w8vz[:], in_=w8vzd[:])
            x8 = xp.tile([P, DT, NLOC], f8, tag="x8")
            XC = 1024
            for ci in range(NLOC // XC):
                c0 = ci * XC
                for kd in range(DT):
                    q = nc.sync if kd % 2 == 0 else nc.scalar
                    q.dma_start(
                        out=x8[:, kd, c0:c0 + XC],
                        in_=x8d[kd * P:(kd + 1) * P, c0:c0 + XC],
                    )
                if ci == 1:
                    wt = wp.tile([P, DT, K], bf16, tag="wt")
                    nc.scalar.dma_start(out=wt[:], in_=wttd[:])
            ones_row = wp.tile([1, P], f32, tag="ones_row")
            nc.vector.memset(ones_row[:], 1.0)

            ut = [up.tile([P, NLOC], bf16, tag=f"ut{h}", name=f"ut{h}") for h in range(2)]
            csu_cols = [accp.tile([P, NB], f32, tag=f"csuc{h}", name=f"csuc{h}") for h in range(2)]

            def load_xb(ib):
                xbt = xbp.tile([P, DT, IB], bf16, tag="xb", name=f"xb{ib}")
                nc.gpsimd.dma_start(
                    out=xbt[:],
                    in_=xbd[:, ib * IB:(ib + 1) * IB].rearrange("(k p) c -> p k c", p=P),
                )
                return xbt

            def t_pass(ib, xbt, wsrc):
                """T = relu(x @ Wt): 4 row-subtiles, one batched out-DMA."""
                for g in range(NSUB // 2):
                    otb = ob.tile([P, 2, K], f32, tag="ob")
                    for s2 in range(2):
                        s = g * 2 + s2
                        pt = ps.tile([P, K], f32, tag="work")
                        for kd in range(DT):
                            nc.tensor.matmul(
                                pt[:], xbt[:, kd, s * P:(s + 1) * P],
                                wsrc[:, kd, :],
                                start=(kd == 0), stop=(kd == DT - 1),
                            )
                        nc.vector.tensor_relu(otb[:, s2, :], pt[:])
                    i0 = ib * IB + g * 2 * P
                    nc.sync.dma_start(
                        out=out[i0:i0 + 2 * P, K:2 * K].rearrange(
                            "(s p) c -> p s c", p=P),
                        in_=otb[:],
                    )

            # ---- phase 1a: V|Z fp8 projection + V^T@Z PSUM chains ----
            # vz col 512 = 1.0 rides the V^T@Z matmul to produce 32*csV in
            # column 256 of the [k, k+1] chain.
            pzh = [ps2.tile([P, K + 1], f32, tag=f"pz{h}", name=f"pz{h}") for h in range(2)]

            def vtz(ib, vz_tiles):
                for h in range(2):
                    for sp in range(NSUB // 2):
                        nc.tensor.matmul(
                            pzh[h][:], vz_tiles[sp][:, :, h * P:(h + 1) * P],
                            vz_tiles[sp][:, :, K:2 * K + 1],
                            start=(ib == 0 and sp == 0),
                            stop=(ib == NB - 1 and sp == NSUB // 2 - 1),
                            perf_mode=DR,
                        )

            prev_vz = None
            for ib in range(NB):
                vz_tiles = []
                for sp in range(NSUB // 2):
                    vzt = vzp.tile([P, 2, 2 * K + 16], f8, tag="vz")
                    for s2 in range(2):
                        s = sp * 2 + s2
                        pvz = ps.tile([P, IB], f32, tag="work")
                        for k2 in range(KD2):
                            nc.tensor.matmul(
                                pvz[:],
                                x8[:, 2 * k2:2 * k2 + 2,
                                   ib * IB + s * P:ib * IB + (s + 1) * P],
                                w8vz[:, 2 * k2:2 * k2 + 2, :],
                                start=(k2 == 0), stop=(k2 == KD2 - 1 and not KODD),
                                perf_mode=DR,
                            )
                        if KODD:
                            nc.tensor.matmul(
                                pvz[:],
                                x8[:, DT - 1, ib * IB + s * P:ib * IB + (s + 1) * P],
                                w8vz[:, DT - 1, :],
                                start=False, stop=True,
                            )
                        nc.vector.tensor_scalar(
                            out=vzt[:, s2, 0:2 * K], in0=pvz[:],
                            scalar1=S_V * SCL, scalar2=0.0,
                            op0=mybir.AluOpType.mult, op1=mybir.AluOpType.max,
                        )
                    nc.vector.memset(vzt[:, :, 2 * K:2 * K + 1], 1.0)
                    vz_tiles.append(vzt)
                if prev_vz is not None:
                    vtz(ib - 1, prev_vz)
                prev_vz = vz_tiles
            vtz(NB - 1, prev_vz)

            # ---- AllReduce A: the full V^T@Z | csV payload ----
            bin_a = dram.tile([2 * P, K + 1], f32)
            bout_a = dram.tile([2 * P, K + 1], f32)
            vtzs = [accp.tile([P, K + 1], f32, tag=f"vtzs{h}", name=f"vtzs{h}") for h in range(2)]
            for h in range(2):
                nc.vector.tensor_copy(vtzs[h][:], pzh[h][:])
                nc.scalar.dma_start(out=bin_a[h * P:(h + 1) * P, :], in_=vtzs[h][:])
            nc.gpsimd.collective_compute(
                "AllReduce", mybir.AluOpType.add,
                replica_groups=[list(range(NCORES))],
                ins=[bin_a.opt()], outs=[bout_a.opt()],
            )

            # ---- phase 1b: all U-passes (fp8 DR), then AllReduce C (csu) ----
            for ib in range(NB):
                for h in range(2):
                    pu = ps.tile([P, IB], f32, tag="work")
                    for k2 in range(KD2):
                        nc.tensor.matmul(
                            pu[:], w8u[:, 2 * k2:2 * k2 + 2, h * P:(h + 1) * P],
                            x8[:, 2 * k2:2 * k2 + 2, ib * IB:(ib + 1) * IB],
                            start=(k2 == 0), stop=(k2 == KD2 - 1 and not KODD),
                            perf_mode=DR,
                        )
                    if KODD:
                        nc.tensor.matmul(
                            pu[:], w8u[:, DT - 1, h * P:(h + 1) * P],
                            x8[:, DT - 1, ib * IB:(ib + 1) * IB],
                            start=False, stop=True,
                        )
                    nc.scalar.activation(
                        ut[h][:, ib * IB:(ib + 1) * IB], pu[:], RELU, scale=SCL,
                        accum_out=csu_cols[h][:, ib:ib + 1],
                    )

            csu = [accp.tile([P, 1], f32, tag=f"csu{h}", name=f"csu{h}") for h in range(2)]
            for h in range(2):
                nc.vector.reduce_sum(csu[h][:], csu_cols[h][:], axis=mybir.AxisListType.X)
            bin_c = dram.tile([2, P], f32)
            bout_c = dram.tile([2, P], f32)
            for h in range(2):
                nc.scalar.dma_start(
                    out=bin_c[h, 0:P].rearrange("(p one) -> p one", one=1),
                    in_=csu[h][:],
                )
            nc.gpsimd.collective_compute(
                "AllReduce", mybir.AluOpType.add,
                replica_groups=[list(range(NCORES))],
                ins=[bin_c.opt()], outs=[bout_c.opt()],
            )
            # Gated copy of the T-weights (gate==1.0 exactly, from csu): the
            # deferred T matmuls read wt2, pinning them after C's launch so
            # they fill C's flight time instead of being hoisted earlier.
            gate = accp.tile([P, 1], f32, tag="gate")
            nc.vector.tensor_scalar(
                out=gate[:], in0=csu[0][:], scalar1=0.0, scalar2=1.0,
                op0=mybir.AluOpType.mult, op1=mybir.AluOpType.add,
            )
            wt2 = wp.tile([P, DT, K], bf16, tag="wt2")
            nc.vector.tensor_scalar_mul(wt2[:], wt[:], gate[:])

            # ---- T-passes (xb streamed per block; last TDEF read wt2) ----
            for ib in range(NB):
                t_pass(ib, load_xb(ib), wt if ib < NB - TDEF else wt2)

            # vtzr = V^T Z (unscaled, bf16) — needs only AllReduce A. Built
            # after the T loop so the DVE queue never head-of-line blocks on
            # A's completion while T drains are pending.
            vtzf = [accp.tile([P, K + 1], f32, tag=f"vtzf{h}", name=f"vtzf{h}") for h in range(2)]
            for h in range(2):
                nc.scalar.dma_start(out=vtzf[h][:], in_=bout_a[h * P:(h + 1) * P, :])
            vtzr = [accp.tile([P, K], bf16, tag=f"vtzr{h}", name=f"vtzr{h}") for h in range(2)]
            for h in range(2):
                nc.vector.tensor_scalar_mul(vtzr[h][:], vtzf[h][:, 0:K], SCL)

            # ---- phase 3: D = 1/(csU.csV/n + eps) as a drain-time scale ----
            csut = accp.tile([P, 2], f32, tag="csut")
            nc.scalar.dma_start(out=csut[:], in_=bout_c.rearrange("t p -> p t"))
            csvt = accp.tile([P, 2], f32, tag="csvt")
            for h in range(2):
                nc.vector.tensor_copy(csvt[:, h:h + 1], vtzf[h][:, K:K + 1])
            pdot = ps.tile([1, 1], f32, tag="work")
            for h in range(2):
                nc.tensor.matmul(
                    pdot[:], csut[:, h:h + 1], csvt[:, h:h + 1],
                    start=(h == 0), stop=(h == 1),
                )
            dsb = accp.tile([1, 1], f32, tag="dsb")
            nc.vector.tensor_scalar(
                out=dsb[:], in0=pdot[:], scalar1=1.0 / (S_V * N_ROWS), scalar2=EPS,
                op0=mybir.AluOpType.mult, op1=mybir.AluOpType.add,
            )
            nc.vector.reciprocal(dsb[:], dsb[:])
            pb = ps.tile([P, 1], f32, tag="work")
            nc.tensor.matmul(pb[:], ones_row[:], dsb[:], start=True, stop=True)
            dbc = accp.tile([P, 1], f32, tag="dbc")
            nc.vector.tensor_copy(dbc[:], pb[:])

            # ---- phase 4: res = (U @ VtZ) * D, batched row-natural writes ----
            # h-major over groups of 4 PSUM tiles: the moving operand stays
            # fixed for the group and each start/stop pair is spread apart,
            # keeping the weight path warm. D lands at the drains (AP scale).
            GG = 4
            for gb in range(NLOC // P // GG):
                prs = [ps.tile([P, K], f32, tag="work", name=f"pr{t}") for t in range(GG)]
                for h in range(2):
                    for t in range(GG):
                        i0 = (gb * GG + t) * P
                        nc.tensor.matmul(
                            prs[t][:], ut[h][:, i0:i0 + P], vtzr[h][:],
                            start=(h == 0), stop=(h == 1),
                        )
                for g2 in range(GG // 2):
                    orb = ob.tile([P, 2, K], f32, tag="ob")
                    for s2 in range(2):
                        t = g2 * 2 + s2
                        # split PSUM->SBUF scaled copies across DVE and ACT
                        if s2 == 0:
                            nc.vector.tensor_scalar_mul(orb[:, s2, :], prs[t][:], dbc[:])
                        else:
                            nc.scalar.mul(orb[:, s2, :], prs[t][:], dbc[:])
                    i0 = (gb * GG + g2 * 2) * P
                    oq = nc.sync if (gb + g2) % 2 == 0 else nc.scalar
                    oq.dma_start(
                        out=out[i0:i0 + 2 * P, 0:K].rearrange(
                            "(s p) c -> p s c", p=P),
                        in_=orb[:],
                    )

    nc.compile()
    return nc


def _get_nc(d_rows):
    if d_rows not in _built:
        _built[d_rows] = _build(d_rows)
    return _built[d_rows]


def _q8(a, s):
    return np.clip(a * s, -240.0, 240.0).astype(E4)


def _run(x, W, b, trace=False, trace_cores=None):
    from concourse.bass_utils import run_bass_kernel_spmd

    x = np.ascontiguousarray(x, dtype=np.float32)
    W = np.ascontiguousarray(W, dtype=np.float32)
    b = np.asarray(b, dtype=np.float32)
    if np.any(b):
        d_rows = 1152  # pad contraction: extra ones-row in x picks up b from W
        WT_full = np.zeros((d_rows, 4 * K), np.float32)
        WT_full[:D_IN] = W.T
        WT_full[D_IN] = b
    else:
        d_rows = D_IN
        WT_full = np.ascontiguousarray(W.T)
    DT = d_rows // P
    w8u = np.ascontiguousarray(
        _q8(WT_full[:, 0:K], S_W).reshape(DT, P, K).transpose(1, 0, 2))
    w8vz = np.ascontiguousarray(
        _q8(WT_full[:, K:3 * K], S_W).reshape(DT, P, 2 * K).transpose(1, 0, 2))
    wtt = np.ascontiguousarray(
        WT_full[:, 3 * K:].astype(BF16).reshape(DT, P, K).transpose(1, 0, 2))
    nc = _get_nc(d_rows)
    in_maps = []
    for c in range(NCORES):
        xs = x[c * NLOC:(c + 1) * NLOC]
        if d_rows == D_IN:
            xTs = np.ascontiguousarray(xs.T)
        else:
            xTs = np.zeros((d_rows, NLOC), np.float32)
            xTs[:D_IN] = xs.T
            xTs[D_IN] = 1.0
        xb_bf = xTs.astype(BF16)
        x8_ = _q8(xb_bf.astype(np.float32), S_X)
        in_maps.append({"x8": x8_, "xb": xb_bf, "w8u": w8u, "w8vz": w8vz, "wtt": wtt})
    res = run_bass_kernel_spmd(
        nc, in_maps, list(range(NCORES)),
        trace=trace, **({"trace_cores": trace_cores} if trace_cores else {}),
    )
    full = np.concatenate([res.results[c]["out"] for c in range(NCORES)], axis=0)
    return full, res


def kernel(x, W, b):
    full, _ = _run(x, W, b)
    return full


# revision 11
# speedup vs baseline: 1.2080x; 1.0389x over previous
"""Low-rank attention Trainium2 kernel (8 NeuronCores, SPMD), fp8 edition.

Math (reference):
    tmp = relu(x @ W.T + b); U,V,Z,T = split(tmp, 4, axis=1)
    norm = sum(U @ colsum(V)) / n + eps ;  D = 1/norm
    out = concat[(U @ (V.T @ Z)) * D, T]

Sharding: rows of x across 8 cores. Per-core partials (V.T@Z [k,k],
colsum(V), colsum(U)) are AllReduced on-device; each core then computes
its local U @ (VtZ) * D.

fp8 design (vs the 339us bf16 baseline):
- U, V, Z projections and V^T@Z run as fp8e4 DoubleRow matmuls (2 k-tiles
  per instruction, measured 216ns steady for moving-512 = true 2x bf16;
  LDWEIGHTS hides behind the previous matmul's streaming).
- x is quantized to fp8 on the HOST (x8 = e4m3(16x), 8MB/core, resident);
  on-device bf16->fp8 converts are not viable (only DVE writes fp8 fast).
- The T block stays bf16 (its error hits the output directly; fp8's ~2.5%
  elementwise would eat the whole 2e-2 budget). bf16 x streams through a
  rolling pool, one [1024, 512] block per T-pass block.
- Scales: x8 = 16x, W8 = 64W, vz fp8 = 32*[V|Z]; U drains unscaled (bf16,
  ACT relu scale 1/1024 + csu accum); VtZ psum = 1024 V^T Z | 32 csV.

Collective hiding (the bf16 baseline exposed ~41us of AllReduce):
- Phase 1a: V|Z + V^T@Z for ALL i-blocks first; V^T@Z accumulates across
  blocks in two PSUM chains (no per-block DVE adds). AllReduce A (the
  whole [k,k+1] x 2 payload) launches at ~1/3 of the kernel.
- Phase 1b: all U-passes; then AllReduce C (csu only, 1KB).
- T-passes follow (last TDEF read a csu-gated copy of the T-weights,
  pinning them after C's launch); phase-4 matmuls need only A's result
  (vtzr = V^T Z unscaled); the data-dependent D = 1/norm is applied at
  the phase-4 PSUM drains as a per-partition AP scale, so C's latency
  hides under the T-pass + phase-4 matmuls.
- DMA rings: x8 + T-out on sync, weights + staging + res-out on scalar,
  xb blocks + collectives on gpsimd.
"""
import sys

sys.path.insert(0, "/opt/trn_rl_repo")
import numpy as np
import ml_dtypes

BF16 = ml_dtypes.bfloat16
E4 = ml_dtypes.float8_e4m3

NCORES = 8
N_ROWS, D_IN, K = 65536, 1024, 256
NLOC = N_ROWS // NCORES      # 8192 rows per core
P = 128
IB = 512                     # i-block width
NB = NLOC // IB              # 16 blocks
EPS = 1e-6
TDEF = 6                     # T-pass blocks deferred behind AllReduce C
S_X, S_W, S_V = 16.0, 64.0, 32.0
X8CHUNKS = [(0, 512), (512, 512), (1024, 1024), (2048, 2048), (4096, 4096)]

_built = {}


def _build(d_rows):
    import concourse.bacc as bacc
    import concourse.mybir as mybir
    import concourse.tile as tile

    dt = mybir.dt
    f32, bf16, f8 = dt.float32, dt.bfloat16, dt.float8e4
    RELU = mybir.ActivationFunctionType.Relu
    DR = mybir.MatmulPerfMode.DoubleRow
    DT = d_rows // P
    KD2 = DT // 2            # DoubleRow kd-pairs
    KODD = DT % 2            # leftover plain-fp8 k-tile (bias-pad path)
    NSUB = IB // P
    SCL = 1.0 / (S_X * S_W)  # psum -> true pre-activation

    nc = bacc.Bacc("TRN2", target_bir_lowering=False, debug=False, num_devices=NCORES)
    x8d = nc.dram_tensor("x8", [d_rows, NLOC], f8, kind="ExternalInput")
    xbd = nc.dram_tensor("xb", [d_rows, NLOC], bf16, kind="ExternalInput")
    w8ud = nc.dram_tensor("w8u", [P, DT, K], f8, kind="ExternalInput")
    w8vzd = nc.dram_tensor("w8vz", [P, DT, 2 * K], f8, kind="ExternalInput")
    wttd = nc.dram_tensor("wtt", [P, DT, K], bf16, kind="ExternalInput")
    out = nc.dram_tensor("out", [NLOC, 2 * K], f32, kind="ExternalOutput")

    with tile.TileContext(nc) as tc:
        with (
            tc.tile_pool(name="wp", bufs=1) as wp,
            tc.tile_pool(name="xp", bufs=1) as xp,
            tc.tile_pool(name="xbp", bufs=8) as xbp,
            tc.tile_pool(name="up", bufs=1) as up,
            tc.tile_pool(name="vzp", bufs=6) as vzp,
            tc.tile_pool(name="ob", bufs=5) as ob,
            tc.tile_pool(name="acc", bufs=1) as accp,
            tc.tile_pool(name="ps", bufs=6, space="PSUM") as ps,
            tc.tile_pool(name="ps2", bufs=1, space="PSUM") as ps2,
            tc.tile_pool(name="dram", bufs=1, space="DRAM") as dram,
        ):
            # Weights (gpsimd/scalar rings), then resident x8 in per-kd
            # column chunks (sync ring, small first chunks so ib0 starts
            # early). bf16 x streams per-block via xbp below.
            w8u = wp.tile([P, DT, K], f8, tag="w8u")
            nc.gpsimd.dma_start(out=w8u[:], in_=w8ud[:])
            w8vz = wp.tile([P, DT, 2 * K], f8, tag="w8vz")
            nc.scalar.dma_start(out=w8vz[:], in_=w8vzd[:])
            x8 = xp.tile([P, DT, NLOC], f8, tag="x8")
            XC = 1024
            for ci in range(NLOC // XC):
                c0 = ci * XC
                for kd in range(DT):
                    q = nc.sync if kd % 2 == 0 else nc.scalar
                    q.dma_start(
                        out=x8[:, kd, c0:c0 + XC],
                        in_=x8d[kd * P:(kd + 1) * P, c0:c0 + XC],
                    )
                if ci == 1:
                    wt = wp.tile([P, DT, K], bf16, tag="wt")
                    nc.scalar.dma_start(out=wt[:], in_=wttd[:])
            ones_row = wp.tile([1, P], f32, tag="ones_row")
            nc.vector.memset(ones_row[:], 1.0)

            ut = [up.tile([P, NLOC], bf16, tag=f"ut{h}", name=f"ut{h}") for h in range(2)]
            csu_cols = [accp.tile([P, NB], f32, tag=f"csuc{h}", name=f"csuc{h}") for h in range(2)]

            def load_xb(ib):
                xbt = xbp.tile([P, DT, IB], bf16, tag="xb", name=f"xb{ib}")
                q = nc.gpsimd if ib % 2 == 0 else nc.scalar
                q.dma_start(
                    out=xbt[:],
                    in_=xbd[:, ib * IB:(ib + 1) * IB].rearrange("(k p) c -> p k c", p=P),
                )
                return xbt

            # Prefetch the first xbp-pool-worth of T-pass x blocks NOW: their
            # triggers must precede the collective triggers on the gpsimd
            # queue, or no xb prefetch happens until both collectives stage.
            xbts = {ib: load_xb(ib) for ib in range(8)}

            def t_pass(ib, xbt, wsrc):
                """T = relu(x @ Wt): 4 row-subtiles, one batched out-DMA."""
                for g in range(NSUB // 2):
                    otb = ob.tile([P, 2, K], f32, tag="ob")
                    for s2 in range(2):
                        s = g * 2 + s2
                        pt = ps.tile([P, K], f32, tag="work")
                        for kd in range(DT):
                            nc.tensor.matmul(
                                pt[:], xbt[:, kd, s * P:(s + 1) * P],
                                wsrc[:, kd, :],
                                start=(kd == 0), stop=(kd == DT - 1),
                            )
                        nc.vector.tensor_relu(otb[:, s2, :], pt[:])
                    i0 = ib * IB + g * 2 * P
                    nc.sync.dma_start(
                        out=out[i0:i0 + 2 * P, K:2 * K].rearrange(
                            "(s p) c -> p s c", p=P),
                        in_=otb[:],
                    )

            # ---- phase 1a: V|Z fp8 projection + V^T@Z PSUM chains ----
            # vz col 512 = 1.0 rides the V^T@Z matmul to produce 32*csV in
            # column 256 of the [k, k+1] chain.
            pzh = [ps2.tile([P, K + 1], f32, tag=f"pz{h}", name=f"pz{h}") for h in range(2)]

            def vtz(ib, vz_tiles):
                for h in range(2):
                    for sp in range(NSUB // 2):
                        nc.tensor.matmul(
                            pzh[h][:], vz_tiles[sp][:, :, h * P:(h + 1) * P],
                            vz_tiles[sp][:, :, K:2 * K + 1],
                            start=(ib == 0 and sp == 0),
                            stop=(ib == NB - 1 and sp == NSUB // 2 - 1),
                            perf_mode=DR,
                        )

            prev_vz = None
            for ib in range(NB):
                vz_tiles = []
                for sp in range(NSUB // 2):
                    vzt = vzp.tile([P, 2, 2 * K + 16], f8, tag="vz")
                    for s2 in range(2):
                        s = sp * 2 + s2
                        pvz = ps.tile([P, IB], f32, tag="work")
                        for k2 in range(KD2):
                            nc.tensor.matmul(
                                pvz[:],
                                x8[:, 2 * k2:2 * k2 + 2,
                                   ib * IB + s * P:ib * IB + (s + 1) * P],
                                w8vz[:, 2 * k2:2 * k2 + 2, :],
                                start=(k2 == 0), stop=(k2 == KD2 - 1 and not KODD),
                                perf_mode=DR,
                            )
                        if KODD:
                            nc.tensor.matmul(
                                pvz[:],
                                x8[:, DT - 1, ib * IB + s * P:ib * IB + (s + 1) * P],
                                w8vz[:, DT - 1, :],
                                start=False, stop=True,
                            )
                        nc.vector.tensor_scalar(
                            out=vzt[:, s2, 0:2 * K], in0=pvz[:],
                            scalar1=S_V * SCL, scalar2=0.0,
                            op0=mybir.AluOpType.mult, op1=mybir.AluOpType.max,
                        )
                    nc.vector.memset(vzt[:, :, 2 * K:2 * K + 1], 1.0)
                    vz_tiles.append(vzt)
                if prev_vz is not None:
                    vtz(ib - 1, prev_vz)
                prev_vz = vz_tiles
            vtz(NB - 1, prev_vz)

            # ---- AllReduce A: the full V^T@Z | csV payload ----
            bin_a = dram.tile([2 * P, K + 1], f32)
            bout_a = dram.tile([2 * P, K + 1], f32)
            vtzs = [accp.tile([P, K + 1], f32, tag=f"vtzs{h}", name=f"vtzs{h}") for h in range(2)]
            for h in range(2):
                nc.vector.tensor_copy(vtzs[h][:], pzh[h][:])
                nc.scalar.dma_start(out=bin_a[h * P:(h + 1) * P, :], in_=vtzs[h][:])
            nc.gpsimd.collective_compute(
                "AllReduce", mybir.AluOpType.add,
                replica_groups=[list(range(NCORES))],
                ins=[bin_a.opt()], outs=[bout_a.opt()],
            )

            # ---- phase 1b: all U-passes (fp8 DR), then AllReduce C (csu) ----
            for ib in range(NB):
                for h in range(2):
                    pu = ps.tile([P, IB], f32, tag="work")
                    for k2 in range(KD2):
                        nc.tensor.matmul(
                            pu[:], w8u[:, 2 * k2:2 * k2 + 2, h * P:(h + 1) * P],
                            x8[:, 2 * k2:2 * k2 + 2, ib * IB:(ib + 1) * IB],
                            start=(k2 == 0), stop=(k2 == KD2 - 1 and not KODD),
                            perf_mode=DR,
                        )
                    if KODD:
                        nc.tensor.matmul(
                            pu[:], w8u[:, DT - 1, h * P:(h + 1) * P],
                            x8[:, DT - 1, ib * IB:(ib + 1) * IB],
                            start=False, stop=True,
                        )
                    nc.scalar.activation(
                        ut[h][:, ib * IB:(ib + 1) * IB], pu[:], RELU, scale=SCL,
                        accum_out=csu_cols[h][:, ib:ib + 1],
                    )

            csu = [accp.tile([P, 1], f32, tag=f"csu{h}", name=f"csu{h}") for h in range(2)]
            for h in range(2):
                nc.vector.reduce_sum(csu[h][:], csu_cols[h][:], axis=mybir.AxisListType.X)
            bin_c = dram.tile([2, P], f32)
            bout_c = dram.tile([2, P], f32)
            for h in range(2):
                nc.scalar.dma_start(
                    out=bin_c[h, 0:P].rearrange("(p one) -> p one", one=1),
                    in_=csu[h][:],
                )
            nc.gpsimd.collective_compute(
                "AllReduce", mybir.AluOpType.add,
                replica_groups=[list(range(NCORES))],
                ins=[bin_c.opt()], outs=[bout_c.opt()],
            )
            # Gated copy of the T-weights (gate==1.0 exactly, from csu): the
            # deferred T matmuls read wt2, pinning them after C's launch so
            # they fill C's flight time instead of being hoisted earlier.
            gate = accp.tile([P, 1], f32, tag="gate")
            nc.vector.tensor_scalar(
                out=gate[:], in0=csu[0][:], scalar1=0.0, scalar2=1.0,
                op0=mybir.AluOpType.mult, op1=mybir.AluOpType.add,
            )
            wt2 = wp.tile([P, DT, K], bf16, tag="wt2")
            nc.vector.tensor_scalar_mul(wt2[:], wt[:], gate[:])

            # ---- T-passes (xb streamed per block; last TDEF read wt2) ----
            for ib in range(NB):
                xbt = xbts.pop(ib) if ib in xbts else load_xb(ib)
                t_pass(ib, xbt, wt if ib < NB - TDEF else wt2)

            # vtzr = V^T Z (unscaled, bf16) — needs only AllReduce A. Built
            # after the T loop so the DVE queue never head-of-line blocks on
            # A's completion while T drains are pending.
            vtzf = [accp.tile([P, K + 1], f32, tag=f"vtzf{h}", name=f"vtzf{h}") for h in range(2)]
            for h in range(2):
                nc.scalar.dma_start(out=vtzf[h][:], in_=bout_a[h * P:(h + 1) * P, :])
            vtzr = [accp.tile([P, K], bf16, tag=f"vtzr{h}", name=f"vtzr{h}") for h in range(2)]
            for h in range(2):
                nc.vector.tensor_scalar_mul(vtzr[h][:], vtzf[h][:, 0:K], SCL)

            # ---- phase 3: D = 1/(csU.csV/n + eps) as a drain-time scale ----
            csut = accp.tile([P, 2], f32, tag="csut")
            nc.scalar.dma_start(out=csut[:], in_=bout_c.rearrange("t p -> p t"))
            csvt = accp.tile([P, 2], f32, tag="csvt")
            for h in range(2):
                nc.vector.tensor_copy(csvt[:, h:h + 1], vtzf[h][:, K:K + 1])
            pdot = ps.tile([1, 1], f32, tag="work")
            for h in range(2):
                nc.tensor.matmul(
                    pdot[:], csut[:, h:h + 1], csvt[:, h:h + 1],
                    start=(h == 0), stop=(h == 1),
                )
            dsb = accp.tile([1, 1], f32, tag="dsb")
            nc.vector.tensor_scalar(
                out=dsb[:], in0=pdot[:], scalar1=1.0 / (S_V * N_ROWS), scalar2=EPS,
                op0=mybir.AluOpType.mult, op1=mybir.AluOpType.add,
            )
            nc.vector.reciprocal(dsb[:], dsb[:])
            pb = ps.tile([P, 1], f32, tag="work")
            nc.tensor.matmul(pb[:], ones_row[:], dsb[:], start=True, stop=True)
            dbc = accp.tile([P, 1], f32, tag="dbc")
            nc.vector.tensor_copy(dbc[:], pb[:])

            # ---- phase 4: res = (U @ VtZ) * D, batched row-natural writes ----
            # h-major over groups of 4 PSUM tiles: the moving operand stays
            # fixed for the group and each start/stop pair is spread apart,
            # keeping the weight path warm. D lands at the drains (AP scale).
            GG = 4
            for gb in range(NLOC // P // GG):
                prs = [ps.tile([P, K], f32, tag="work", name=f"pr{t}") for t in range(GG)]
                for h in range(2):
                    for t in range(GG):
                        i0 = (gb * GG + t) * P
                        nc.tensor.matmul(
                            prs[t][:], ut[h][:, i0:i0 + P], vtzr[h][:],
                            start=(h == 0), stop=(h == 1),
                        )
                for g2 in range(GG // 2):
                    orb = ob.tile([P, 2, K], f32, tag="ob")
                    for s2 in range(2):
                        t = g2 * 2 + s2
                        # split PSUM->SBUF scaled copies across DVE and ACT
                        if s2 == 0:
                            nc.vector.tensor_scalar_mul(orb[:, s2, :], prs[t][:], dbc[:])
                        else:
                            nc.scalar.mul(orb[:, s2, :], prs[t][:], dbc[:])
                    i0 = (gb * GG + g2 * 2) * P
                    oq = nc.sync if (gb + g2) % 2 == 0 else nc.scalar
                    oq.dma_start(
                        out=out[i0:i0 + 2 * P, 0:K].rearrange(
                            "(s p) c -> p s c", p=P),
                        in_=orb[:],
                    )

    nc.compile()
    return nc


def _get_nc(d_rows):
    if d_rows not in _built:
        _built[d_rows] = _build(d_rows)
    return _built[d_rows]


def _q8(a, s):
    return np.clip(a * s, -240.0, 240.0).astype(E4)


def _run(x, W, b, trace=False, trace_cores=None):
    from concourse.bass_utils import run_bass_kernel_spmd

    x = np.ascontiguousarray(x, dtype=np.float32)
    W = np.ascontiguousarray(W, dtype=np.float32)
    b = np.asarray(b, dtype=np.float32)
    if np.any(b):
        d_rows = 1152  # pad contraction: extra ones-row in x picks up b from W
        WT_full = np.zeros((d_rows, 4 * K), np.float32)
        WT_full[:D_IN] = W.T
        WT_full[D_IN] = b
    else:
        d_rows = D_IN
        WT_full = np.ascontiguousarray(W.T)
    DT = d_rows // P
    w8u = np.ascontiguousarray(
        _q8(WT_full[:, 0:K], S_W).reshape(DT, P, K).transpose(1, 0, 2))
    w8vz = np.ascontiguousarray(
        _q8(WT_full[:, K:3 * K], S_W).reshape(DT, P, 2 * K).transpose(1, 0, 2))
    wtt = np.ascontiguousarray(
        WT_full[:, 3 * K:].astype(BF16).reshape(DT, P, K).transpose(1, 0, 2))
    nc = _get_nc(d_rows)
    in_maps = []
    for c in range(NCORES):
        xs = x[c * NLOC:(c + 1) * NLOC]
        if d_rows == D_IN:
            xTs = np.ascontiguousarray(xs.T)
        else:
            xTs = np.zeros((d_rows, NLOC), np.float32)
            xTs[:D_IN] = xs.T
            xTs[D_IN] = 1.0
        xb_bf = xTs.astype(BF16)
        x8_ = _q8(xb_bf.astype(np.float32), S_X)
        in_maps.append({"x8": x8_, "xb": xb_bf, "w8u": w8u, "w8vz": w8vz, "wtt": wtt})
    res = run_bass_kernel_spmd(
        nc, in_maps, list(range(NCORES)),
        trace=trace, **({"trace_cores": trace_cores} if trace_cores else {}),
    )
    full = np.concatenate([res.results[c]["out"] for c in range(NCORES)], axis=0)
    return full, res


def kernel(x, W, b):
    full, _ = _run(x, W, b)
    return full


# revision 12
# speedup vs baseline: 1.2493x; 1.0342x over previous
"""Low-rank attention Trainium2 kernel (8 NeuronCores, SPMD), fp8 edition.

Math (reference):
    tmp = relu(x @ W.T + b); U,V,Z,T = split(tmp, 4, axis=1)
    norm = sum(U @ colsum(V)) / n + eps ;  D = 1/norm
    out = concat[(U @ (V.T @ Z)) * D, T]

Sharding: rows of x across 8 cores. Per-core partials (V.T@Z [k,k],
colsum(V), colsum(U)) are AllReduced on-device; each core then computes
its local U @ (VtZ) * D.

fp8 design (vs the 339us bf16 baseline):
- U, V, Z projections and V^T@Z run as fp8e4 DoubleRow matmuls (2 k-tiles
  per instruction, measured 216ns steady for moving-512 = true 2x bf16;
  LDWEIGHTS hides behind the previous matmul's streaming).
- x is quantized to fp8 on the HOST (x8 = e4m3(16x), 8MB/core, resident);
  on-device bf16->fp8 converts are not viable (only DVE writes fp8 fast).
- The T block stays bf16 (its error hits the output directly; fp8's ~2.5%
  elementwise would eat the whole 2e-2 budget). bf16 x streams through a
  rolling pool, one [1024, 512] block per T-pass block.
- Scales: x8 = 16x, W8 = 64W, vz fp8 = 32*[V|Z]; U drains unscaled (bf16,
  ACT relu scale 1/1024 + csu accum); VtZ psum = 1024 V^T Z | 32 csV.

Collective hiding (the bf16 baseline exposed ~41us of AllReduce):
- Phase 1a: V|Z + V^T@Z for ALL i-blocks first; V^T@Z accumulates across
  blocks in two PSUM chains (no per-block DVE adds). AllReduce A (the
  whole [k,k+1] x 2 payload) launches at ~1/3 of the kernel.
- Phase 1b: all U-passes; then AllReduce C (csu only, 1KB).
- T-passes follow (last TDEF read a csu-gated copy of the T-weights,
  pinning them after C's launch); phase-4 matmuls need only A's result
  (vtzr = V^T Z unscaled); the data-dependent D = 1/norm is applied at
  the phase-4 PSUM drains as a per-partition AP scale, so C's latency
  hides under the T-pass + phase-4 matmuls.
- DMA rings: x8 + T-out on sync, weights + staging + res-out on scalar,
  xb blocks + collectives on gpsimd.
"""
import sys

sys.path.insert(0, "/opt/trn_rl_repo")
import numpy as np
import ml_dtypes

BF16 = ml_dtypes.bfloat16
E4 = ml_dtypes.float8_e4m3

NCORES = 8
N_ROWS, D_IN, K = 65536, 1024, 256
NLOC = N_ROWS // NCORES      # 8192 rows per core
P = 128
IB = 512                     # i-block width
NB = NLOC // IB              # 16 blocks
EPS = 1e-6
TDEF = 6                     # T-pass blocks deferred behind AllReduce C
S_X, S_W, S_V = 16.0, 64.0, 32.0
X8CHUNKS = [(0, 512), (512, 512), (1024, 1024), (2048, 2048), (4096, 4096)]

_built = {}


def _build(d_rows):
    import concourse.bacc as bacc
    import concourse.mybir as mybir
    import concourse.tile as tile

    dt = mybir.dt
    f32, bf16, f8 = dt.float32, dt.bfloat16, dt.float8e4
    RELU = mybir.ActivationFunctionType.Relu
    DR = mybir.MatmulPerfMode.DoubleRow
    DT = d_rows // P
    KD2 = DT // 2            # DoubleRow kd-pairs
    KODD = DT % 2            # leftover plain-fp8 k-tile (bias-pad path)
    NSUB = IB // P
    SCL = 1.0 / (S_X * S_W)  # psum -> true pre-activation

    nc = bacc.Bacc("TRN2", target_bir_lowering=False, debug=False, num_devices=NCORES)
    x8d = nc.dram_tensor("x8", [d_rows, NLOC], f8, kind="ExternalInput")
    xbd = nc.dram_tensor("xb", [d_rows, NLOC], bf16, kind="ExternalInput")
    w8ud = nc.dram_tensor("w8u", [P, DT, K], f8, kind="ExternalInput")
    w8vzd = nc.dram_tensor("w8vz", [P, DT, 2 * K], f8, kind="ExternalInput")
    wttd = nc.dram_tensor("wtt", [P, DT, K], bf16, kind="ExternalInput")
    out = nc.dram_tensor("out", [NLOC, 2 * K], f32, kind="ExternalOutput")

    with tile.TileContext(nc) as tc:
        with (
            tc.tile_pool(name="wp", bufs=1) as wp,
            tc.tile_pool(name="xp", bufs=1) as xp,
            tc.tile_pool(name="xbp", bufs=7) as xbp,
            tc.tile_pool(name="up", bufs=1) as up,
            tc.tile_pool(name="vzp", bufs=6) as vzp,
            tc.tile_pool(name="ob", bufs=3) as ob,
            tc.tile_pool(name="acc", bufs=1) as accp,
            tc.tile_pool(name="ps", bufs=6, space="PSUM") as ps,
            tc.tile_pool(name="ps2", bufs=1, space="PSUM") as ps2,
            tc.tile_pool(name="dram", bufs=1, space="DRAM") as dram,
        ):
            # Weights (gpsimd/scalar rings), then resident x8 in per-kd
            # column chunks (sync ring, small first chunks so ib0 starts
            # early). bf16 x streams per-block via xbp below.
            w8u = wp.tile([P, DT, K], f8, tag="w8u")
            nc.gpsimd.dma_start(out=w8u[:], in_=w8ud[:])
            w8vz = wp.tile([P, DT, 2 * K], f8, tag="w8vz")
            nc.gpsimd.dma_start(out=w8vz[:], in_=w8vzd[:])
            ut = [up.tile([P, NLOC], bf16, tag=f"ut{h}", name=f"ut{h}") for h in range(2)]
            xbts = {}

            def load_xb(ib):
                xbt = xbp.tile([P, DT, IB], bf16, tag="xb", name=f"xb{ib}")
                q = nc.sync if ib % 2 == 0 else nc.scalar
                q.dma_start(
                    out=xbt[:],
                    in_=xbd[:, ib * IB:(ib + 1) * IB].rearrange("(k p) c -> p k c", p=P),
                )
                return xbt

            x8 = xp.tile([P, DT, NLOC], f8, tag="x8")
            XC = 1024
            for ci in range(NLOC // XC):
                c0 = ci * XC
                for kd in range(DT):
                    q = nc.sync if kd % 2 == 0 else nc.scalar
                    q.dma_start(
                        out=x8[:, kd, c0:c0 + XC],
                        in_=x8d[kd * P:(kd + 1) * P, c0:c0 + XC],
                    )
                if ci == 1:
                    wt = wp.tile([P, DT, K], bf16, tag="wt")
                    nc.gpsimd.dma_start(out=wt[:], in_=wttd[:])
                if 1 <= ci <= 7:
                    xbts[ci - 1] = load_xb(ci - 1)
            ones_row = wp.tile([1, P], f32, tag="ones_row")
            nc.vector.memset(ones_row[:], 1.0)

            csu_cols = [accp.tile([P, NB], f32, tag=f"csuc{h}", name=f"csuc{h}") for h in range(2)]

            def t_pass(ib, xbt, wsrc):
                """T = relu(x @ Wt): 4 row-subtiles, one batched out-DMA."""
                otb = ob.tile([P, NSUB, K], f32, tag="ob")
                for s in range(NSUB):
                    pt = ps.tile([P, K], f32, tag="work")
                    for kd in range(DT):
                        nc.tensor.matmul(
                            pt[:], xbt[:, kd, s * P:(s + 1) * P],
                            wsrc[:, kd, :],
                            start=(kd == 0), stop=(kd == DT - 1),
                        )
                    nc.vector.tensor_relu(otb[:, s, :], pt[:])
                i0 = ib * IB
                nc.sync.dma_start(
                    out=out[i0:i0 + IB, K:2 * K].rearrange(
                        "(s p) c -> p s c", p=P),
                    in_=otb[:],
                )

            # ---- phase 1a: V|Z fp8 projection + V^T@Z PSUM chains ----
            # vz col 512 = 1.0 rides the V^T@Z matmul to produce 32*csV in
            # column 256 of the [k, k+1] chain.
            pzh = [ps2.tile([P, K + 1], f32, tag=f"pz{h}", name=f"pz{h}") for h in range(2)]

            def vtz(ib, vz_tiles):
                for h in range(2):
                    for sp in range(NSUB // 2):
                        nc.tensor.matmul(
                            pzh[h][:], vz_tiles[sp][:, :, h * P:(h + 1) * P],
                            vz_tiles[sp][:, :, K:2 * K + 1],
                            start=(ib == 0 and sp == 0),
                            stop=(ib == NB - 1 and sp == NSUB // 2 - 1),
                            perf_mode=DR,
                        )

            prev_vz = None
            for ib in range(NB):
                vz_tiles = []
                for sp in range(NSUB // 2):
                    vzt = vzp.tile([P, 2, 2 * K + 16], f8, tag="vz")
                    for s2 in range(2):
                        s = sp * 2 + s2
                        pvz = ps.tile([P, IB], f32, tag="work")
                        for k2 in range(KD2):
                            nc.tensor.matmul(
                                pvz[:],
                                x8[:, 2 * k2:2 * k2 + 2,
                                   ib * IB + s * P:ib * IB + (s + 1) * P],
                                w8vz[:, 2 * k2:2 * k2 + 2, :],
                                start=(k2 == 0), stop=(k2 == KD2 - 1 and not KODD),
                                perf_mode=DR,
                            )
                        if KODD:
                            nc.tensor.matmul(
                                pvz[:],
                                x8[:, DT - 1, ib * IB + s * P:ib * IB + (s + 1) * P],
                                w8vz[:, DT - 1, :],
                                start=False, stop=True,
                            )
                        nc.vector.tensor_scalar(
                            out=vzt[:, s2, 0:2 * K], in0=pvz[:],
                            scalar1=S_V * SCL, scalar2=0.0,
                            op0=mybir.AluOpType.mult, op1=mybir.AluOpType.max,
                        )
                    nc.vector.memset(vzt[:, :, 2 * K:2 * K + 1], 1.0)
                    vz_tiles.append(vzt)
                if prev_vz is not None:
                    vtz(ib - 1, prev_vz)
                prev_vz = vz_tiles
            vtz(NB - 1, prev_vz)

            # ---- AllReduce A: the full V^T@Z | csV payload ----
            bin_a = dram.tile([2 * P, K + 1], f32)
            bout_a = dram.tile([2 * P, K + 1], f32)
            vtzs = [accp.tile([P, K + 1], f32, tag=f"vtzs{h}", name=f"vtzs{h}") for h in range(2)]
            for h in range(2):
                nc.vector.tensor_copy(vtzs[h][:], pzh[h][:])
                nc.scalar.dma_start(out=bin_a[h * P:(h + 1) * P, :], in_=vtzs[h][:])
            nc.gpsimd.collective_compute(
                "AllReduce", mybir.AluOpType.add,
                replica_groups=[list(range(NCORES))],
                ins=[bin_a.opt()], outs=[bout_a.opt()],
            )

            # ---- phase 1b: all U-passes (fp8 DR), then AllReduce C (csu) ----
            for ib in range(NB):
                for h in range(2):
                    pu = ps.tile([P, IB], f32, tag="work")
                    for k2 in range(KD2):
                        nc.tensor.matmul(
                            pu[:], w8u[:, 2 * k2:2 * k2 + 2, h * P:(h + 1) * P],
                            x8[:, 2 * k2:2 * k2 + 2, ib * IB:(ib + 1) * IB],
                            start=(k2 == 0), stop=(k2 == KD2 - 1 and not KODD),
                            perf_mode=DR,
                        )
                    if KODD:
                        nc.tensor.matmul(
                            pu[:], w8u[:, DT - 1, h * P:(h + 1) * P],
                            x8[:, DT - 1, ib * IB:(ib + 1) * IB],
                            start=False, stop=True,
                        )
                    nc.scalar.activation(
                        ut[h][:, ib * IB:(ib + 1) * IB], pu[:], RELU, scale=SCL,
                        accum_out=csu_cols[h][:, ib:ib + 1],
                    )

            csu = [accp.tile([P, 1], f32, tag=f"csu{h}", name=f"csu{h}") for h in range(2)]
            for h in range(2):
                nc.vector.reduce_sum(csu[h][:], csu_cols[h][:], axis=mybir.AxisListType.X)
            bin_c = dram.tile([2, P], f32)
            bout_c = dram.tile([2, P], f32)
            for h in range(2):
                nc.scalar.dma_start(
                    out=bin_c[h, 0:P].rearrange("(p one) -> p one", one=1),
                    in_=csu[h][:],
                )
            nc.gpsimd.collective_compute(
                "AllReduce", mybir.AluOpType.add,
                replica_groups=[list(range(NCORES))],
                ins=[bin_c.opt()], outs=[bout_c.opt()],
            )
            # Gated copy of the T-weights (gate==1.0 exactly, from csu): the
            # deferred T matmuls read wt2, pinning them after C's launch so
            # they fill C's flight time instead of being hoisted earlier.
            gate = accp.tile([P, 1], f32, tag="gate")
            nc.vector.tensor_scalar(
                out=gate[:], in0=csu[0][:], scalar1=0.0, scalar2=1.0,
                op0=mybir.AluOpType.mult, op1=mybir.AluOpType.add,
            )
            wt2 = wp.tile([P, DT, K], bf16, tag="wt2")
            nc.vector.tensor_scalar_mul(wt2[:], wt[:], gate[:])

            # ---- T-passes (xb streamed per block; last TDEF read wt2) ----
            for ib in range(NB):
                xbt = xbts.pop(ib) if ib in xbts else load_xb(ib)
                t_pass(ib, xbt, wt if ib < NB - TDEF else wt2)

            # vtzr = V^T Z (unscaled, bf16) — needs only AllReduce A. Built
            # after the T loop so the DVE queue never head-of-line blocks on
            # A's completion while T drains are pending.
            vtzf = [accp.tile([P, K + 1], f32, tag=f"vtzf{h}", name=f"vtzf{h}") for h in range(2)]
            for h in range(2):
                nc.scalar.dma_start(out=vtzf[h][:], in_=bout_a[h * P:(h + 1) * P, :])
            vtzr = [accp.tile([P, K], bf16, tag=f"vtzr{h}", name=f"vtzr{h}") for h in range(2)]
            for h in range(2):
                nc.vector.tensor_scalar_mul(vtzr[h][:], vtzf[h][:, 0:K], SCL)

            # ---- phase 3: D = 1/(csU.csV/n + eps) as a drain-time scale ----
            csut = accp.tile([P, 2], f32, tag="csut")
            nc.scalar.dma_start(out=csut[:], in_=bout_c.rearrange("t p -> p t"))
            csvt = accp.tile([P, 2], f32, tag="csvt")
            for h in range(2):
                nc.vector.tensor_copy(csvt[:, h:h + 1], vtzf[h][:, K:K + 1])
            pdot = ps.tile([1, 1], f32, tag="work")
            for h in range(2):
                nc.tensor.matmul(
                    pdot[:], csut[:, h:h + 1], csvt[:, h:h + 1],
                    start=(h == 0), stop=(h == 1),
                )
            dsb = accp.tile([1, 1], f32, tag="dsb")
            nc.vector.tensor_scalar(
                out=dsb[:], in0=pdot[:], scalar1=1.0 / (S_V * N_ROWS), scalar2=EPS,
                op0=mybir.AluOpType.mult, op1=mybir.AluOpType.add,
            )
            nc.vector.reciprocal(dsb[:], dsb[:])
            pb = ps.tile([P, 1], f32, tag="work")
            nc.tensor.matmul(pb[:], ones_row[:], dsb[:], start=True, stop=True)
            dbc = accp.tile([P, 1], f32, tag="dbc")
            nc.vector.tensor_copy(dbc[:], pb[:])

            # ---- phase 4: res = (U @ VtZ) * D, batched row-natural writes ----
            # h-major over groups of 4 PSUM tiles: the moving operand stays
            # fixed for the group and each start/stop pair is spread apart,
            # keeping the weight path warm. D lands at the drains (AP scale).
            GG = 4
            for gb in range(NLOC // P // GG):
                prs = [ps.tile([P, K], f32, tag="work", name=f"pr{t}") for t in range(GG)]
                for h in range(2):
                    for t in range(GG):
                        i0 = (gb * GG + t) * P
                        nc.tensor.matmul(
                            prs[t][:], ut[h][:, i0:i0 + P], vtzr[h][:],
                            start=(h == 0), stop=(h == 1),
                        )
                orb = ob.tile([P, GG, K], f32, tag="ob")
                for t in range(GG):
                    # split PSUM->SBUF scaled copies across DVE and ACT
                    if t % 2 == 0:
                        nc.vector.tensor_scalar_mul(orb[:, t, :], prs[t][:], dbc[:])
                    else:
                        nc.scalar.mul(orb[:, t, :], prs[t][:], dbc[:])
                i0 = gb * GG * P
                oq = nc.sync if gb % 2 == 0 else nc.scalar
                oq.dma_start(
                    out=out[i0:i0 + GG * P, 0:K].rearrange(
                        "(s p) c -> p s c", p=P),
                    in_=orb[:],
                )

    nc.compile()
    return nc


def _get_nc(d_rows):
    if d_rows not in _built:
        _built[d_rows] = _build(d_rows)
    return _built[d_rows]


def _q8(a, s):
    return np.clip(a * s, -240.0, 240.0).astype(E4)


def _run(x, W, b, trace=False, trace_cores=None):
    from concourse.bass_utils import run_bass_kernel_spmd

    x = np.ascontiguousarray(x, dtype=np.float32)
    W = np.ascontiguousarray(W, dtype=np.float32)
    b = np.asarray(b, dtype=np.float32)
    if np.any(b):
        d_rows = 1152  # pad contraction: extra ones-row in x picks up b from W
        WT_full = np.zeros((d_rows, 4 * K), np.float32)
        WT_full[:D_IN] = W.T
        WT_full[D_IN] = b
    else:
        d_rows = D_IN
        WT_full = np.ascontiguousarray(W.T)
    DT = d_rows // P
    w8u = np.ascontiguousarray(
        _q8(WT_full[:, 0:K], S_W).reshape(DT, P, K).transpose(1, 0, 2))
    w8vz = np.ascontiguousarray(
        _q8(WT_full[:, K:3 * K], S_W).reshape(DT, P, 2 * K).transpose(1, 0, 2))
    wtt = np.ascontiguousarray(
        WT_full[:, 3 * K:].astype(BF16).reshape(DT, P, K).transpose(1, 0, 2))
    nc = _get_nc(d_rows)
    in_maps = []
    for c in range(NCORES):
        xs = x[c * NLOC:(c + 1) * NLOC]
        if d_rows == D_IN:
            xTs = np.ascontiguousarray(xs.T)
        else:
            xTs = np.zeros((d_rows, NLOC), np.float32)
            xTs[:D_IN] = xs.T
            xTs[D_IN] = 1.0
        xb_bf = xTs.astype(BF16)
        x8_ = _q8(xb_bf.astype(np.float32), S_X)
        in_maps.append({"x8": x8_, "xb": xb_bf, "w8u": w8u, "w8vz": w8vz, "wtt": wtt})
    res = run_bass_kernel_spmd(
        nc, in_maps, list(range(NCORES)),
        trace=trace, **({"trace_cores": trace_cores} if trace_cores else {}),
    )
    full = np.concatenate([res.results[c]["out"] for c in range(NCORES)], axis=0)
    return full, res


def kernel(x, W, b):
    full, _ = _run(x, W, b)
    return full


# revision 13
# speedup vs baseline: 1.2583x; 1.0072x over previous
"""Low-rank attention Trainium2 kernel (8 NeuronCores, SPMD), fp8 edition.

Math (reference):
    tmp = relu(x @ W.T + b); U,V,Z,T = split(tmp, 4, axis=1)
    norm = sum(U @ colsum(V)) / n + eps ;  D = 1/norm
    out = concat[(U @ (V.T @ Z)) * D, T]

Sharding: rows of x across 8 cores. Per-core partials (V.T@Z [k,k],
colsum(V), colsum(U)) are AllReduced on-device; each core then computes
its local U @ (VtZ) * D.

fp8 design (vs the 339us bf16 baseline):
- U, V, Z projections and V^T@Z run as fp8e4 DoubleRow matmuls (2 k-tiles
  per instruction, measured 216ns steady for moving-512 = true 2x bf16;
  LDWEIGHTS hides behind the previous matmul's streaming).
- x is quantized to fp8 on the HOST (x8 = e4m3(16x), 8MB/core, resident);
  on-device bf16->fp8 converts are not viable (only DVE writes fp8 fast).
- The T block stays bf16 (its error hits the output directly; fp8's ~2.5%
  elementwise would eat the whole 2e-2 budget). bf16 x streams through a
  rolling pool, one [1024, 512] block per T-pass block.
- Scales: x8 = 16x, W8 = 64W, vz fp8 = 32*[V|Z]; U drains unscaled (bf16,
  ACT relu scale 1/1024 + csu accum); VtZ psum = 1024 V^T Z | 32 csV.

Collective hiding (the bf16 baseline exposed ~41us of AllReduce):
- Phase 1a: V|Z + V^T@Z for ALL i-blocks first; V^T@Z accumulates across
  blocks in two PSUM chains (no per-block DVE adds). AllReduce A (the
  whole [k,k+1] x 2 payload) launches at ~1/3 of the kernel.
- Phase 1b: all U-passes; then AllReduce C (csu only, 1KB).
- T-passes follow (last TDEF read a csu-gated copy of the T-weights,
  pinning them after C's launch); phase-4 matmuls need only A's result
  (vtzr = V^T Z unscaled); the data-dependent D = 1/norm is applied at
  the phase-4 PSUM drains as a per-partition AP scale, so C's latency
  hides under the T-pass + phase-4 matmuls.
- DMA rings: x8 + T-out on sync, weights + staging + res-out on scalar,
  xb blocks + collectives on gpsimd.
"""
import sys

sys.path.insert(0, "/opt/trn_rl_repo")
import numpy as np
import ml_dtypes

BF16 = ml_dtypes.bfloat16
E4 = ml_dtypes.float8_e4m3

NCORES = 8
N_ROWS, D_IN, K = 65536, 1024, 256
NLOC = N_ROWS // NCORES      # 8192 rows per core
P = 128
IB = 512                     # i-block width
NB = NLOC // IB              # 16 blocks
EPS = 1e-6
TDEF = 6                     # T-pass blocks deferred behind AllReduce C
S_X, S_W, S_V = 16.0, 64.0, 32.0
X8CHUNKS = [(0, 512), (512, 512), (1024, 1024), (2048, 2048), (4096, 4096)]

_built = {}


def _build(d_rows):
    import concourse.bacc as bacc
    import concourse.mybir as mybir
    import concourse.tile as tile

    dt = mybir.dt
    f32, bf16, f8 = dt.float32, dt.bfloat16, dt.float8e4
    RELU = mybir.ActivationFunctionType.Relu
    DR = mybir.MatmulPerfMode.DoubleRow
    DT = d_rows // P
    KD2 = DT // 2            # DoubleRow kd-pairs
    KODD = DT % 2            # leftover plain-fp8 k-tile (bias-pad path)
    NSUB = IB // P
    SCL = 1.0 / (S_X * S_W)  # psum -> true pre-activation

    nc = bacc.Bacc("TRN2", target_bir_lowering=False, debug=False, num_devices=NCORES)
    NBX = NLOC // 1024       # x8 resident-load chunks
    x8d = nc.dram_tensor("x8", [NBX, P, DT, 1024], f8, kind="ExternalInput")
    xbd = nc.dram_tensor("xb", [NB, P, DT, IB], bf16, kind="ExternalInput")
    w8ud = nc.dram_tensor("w8u", [P, DT, K], f8, kind="ExternalInput")
    w8vzd = nc.dram_tensor("w8vz", [P, DT, 2 * K], f8, kind="ExternalInput")
    wttd = nc.dram_tensor("wtt", [P, DT, K], bf16, kind="ExternalInput")
    out = nc.dram_tensor("out", [NLOC, 2 * K], f32, kind="ExternalOutput")

    with tile.TileContext(nc) as tc:
        with (
            tc.tile_pool(name="wp", bufs=1) as wp,
            tc.tile_pool(name="xp", bufs=1) as xp,
            tc.tile_pool(name="xbp", bufs=7) as xbp,
            tc.tile_pool(name="up", bufs=1) as up,
            tc.tile_pool(name="vzp", bufs=6) as vzp,
            tc.tile_pool(name="ob", bufs=3) as ob,
            tc.tile_pool(name="acc", bufs=1) as accp,
            tc.tile_pool(name="ps", bufs=6, space="PSUM") as ps,
            tc.tile_pool(name="ps2", bufs=1, space="PSUM") as ps2,
            tc.tile_pool(name="dram", bufs=1, space="DRAM") as dram,
        ):
            # Weights (gpsimd/scalar rings), then resident x8 in per-kd
            # column chunks (sync ring, small first chunks so ib0 starts
            # early). bf16 x streams per-block via xbp below.
            w8u = wp.tile([P, DT, K], f8, tag="w8u")
            nc.gpsimd.dma_start(out=w8u[:], in_=w8ud[:])
            w8vz = wp.tile([P, DT, 2 * K], f8, tag="w8vz")
            nc.gpsimd.dma_start(out=w8vz[:], in_=w8vzd[:])
            ut = [up.tile([P, NLOC], bf16, tag=f"ut{h}", name=f"ut{h}") for h in range(2)]
            xbts = {}

            def load_xb(ib):
                xbt = xbp.tile([P, DT, IB], bf16, tag="xb", name=f"xb{ib}")
                q = nc.sync if ib % 2 == 0 else nc.scalar
                q.dma_start(out=xbt[:], in_=xbd[ib])
                return xbt

            XC = 1024
            x8 = xp.tile([P, NBX, DT, XC], f8, tag="x8")
            for ci in range(NBX):
                q = nc.sync if ci % 2 == 0 else nc.scalar
                q.dma_start(out=x8[:, ci, :, :], in_=x8d[ci])
                if ci == 1:
                    wt = wp.tile([P, DT, K], bf16, tag="wt")
                    nc.gpsimd.dma_start(out=wt[:], in_=wttd[:])
                if 1 <= ci <= 7:
                    xbts[ci - 1] = load_xb(ci - 1)
            ones_row = wp.tile([1, P], f32, tag="ones_row")
            nc.vector.memset(ones_row[:], 1.0)

            csu_cols = [accp.tile([P, NB], f32, tag=f"csuc{h}", name=f"csuc{h}") for h in range(2)]

            def t_pass(ib, xbt, wsrc):
                """T = relu(x @ Wt): 4 row-subtiles, one batched out-DMA."""
                otb = ob.tile([P, NSUB, K], f32, tag="ob")
                for s in range(NSUB):
                    pt = ps.tile([P, K], f32, tag="work")
                    for kd in range(DT):
                        nc.tensor.matmul(
                            pt[:], xbt[:, kd, s * P:(s + 1) * P],
                            wsrc[:, kd, :],
                            start=(kd == 0), stop=(kd == DT - 1),
                        )
                    nc.vector.tensor_relu(otb[:, s, :], pt[:])
                i0 = ib * IB
                nc.sync.dma_start(
                    out=out[i0:i0 + IB, K:2 * K].rearrange(
                        "(s p) c -> p s c", p=P),
                    in_=otb[:],
                )

            # ---- phase 1a: V|Z fp8 projection + V^T@Z PSUM chains ----
            # vz col 512 = 1.0 rides the V^T@Z matmul to produce 32*csV in
            # column 256 of the [k, k+1] chain.
            pzh = [ps2.tile([P, K + 1], f32, tag=f"pz{h}", name=f"pz{h}") for h in range(2)]

            def vtz(ib, vz_tiles):
                for h in range(2):
                    for sp in range(NSUB // 2):
                        nc.tensor.matmul(
                            pzh[h][:], vz_tiles[sp][:, :, h * P:(h + 1) * P],
                            vz_tiles[sp][:, :, K:2 * K + 1],
                            start=(ib == 0 and sp == 0),
                            stop=(ib == NB - 1 and sp == NSUB // 2 - 1),
                            perf_mode=DR,
                        )

            prev_vz = None
            for ib in range(NB):
                vz_tiles = []
                for sp in range(NSUB // 2):
                    vzt = vzp.tile([P, 2, 2 * K + 16], f8, tag="vz")
                    for s2 in range(2):
                        s = sp * 2 + s2
                        pvz = ps.tile([P, IB], f32, tag="work")
                        ci, off = ib // 2, (ib % 2) * IB + s * P
                        for k2 in range(KD2):
                            nc.tensor.matmul(
                                pvz[:],
                                x8[:, ci, 2 * k2:2 * k2 + 2, off:off + P],
                                w8vz[:, 2 * k2:2 * k2 + 2, :],
                                start=(k2 == 0), stop=(k2 == KD2 - 1 and not KODD),
                                perf_mode=DR,
                            )
                        if KODD:
                            nc.tensor.matmul(
                                pvz[:],
                                x8[:, ci, DT - 1, off:off + P],
                                w8vz[:, DT - 1, :],
                                start=False, stop=True,
                            )
                        nc.vector.tensor_scalar(
                            out=vzt[:, s2, 0:2 * K], in0=pvz[:],
                            scalar1=S_V * SCL, scalar2=0.0,
                            op0=mybir.AluOpType.mult, op1=mybir.AluOpType.max,
                        )
                    nc.vector.memset(vzt[:, :, 2 * K:2 * K + 1], 1.0)
                    vz_tiles.append(vzt)
                if prev_vz is not None:
                    vtz(ib - 1, prev_vz)
                prev_vz = vz_tiles
            vtz(NB - 1, prev_vz)

            # ---- AllReduce A: the full V^T@Z | csV payload ----
            bin_a = dram.tile([2 * P, K + 1], f32)
            bout_a = dram.tile([2 * P, K + 1], f32)
            vtzs = [accp.tile([P, K + 1], f32, tag=f"vtzs{h}", name=f"vtzs{h}") for h in range(2)]
            for h in range(2):
                nc.vector.tensor_copy(vtzs[h][:], pzh[h][:])
                nc.scalar.dma_start(out=bin_a[h * P:(h + 1) * P, :], in_=vtzs[h][:])
            nc.gpsimd.collective_compute(
                "AllReduce", mybir.AluOpType.add,
                replica_groups=[list(range(NCORES))],
                ins=[bin_a.opt()], outs=[bout_a.opt()],
            )

            # ---- phase 1b: all U-passes (fp8 DR), then AllReduce C (csu) ----
            for ib in range(NB):
                for h in range(2):
                    pu = ps.tile([P, IB], f32, tag="work")
                    ci, off = ib // 2, (ib % 2) * IB
                    for k2 in range(KD2):
                        nc.tensor.matmul(
                            pu[:], w8u[:, 2 * k2:2 * k2 + 2, h * P:(h + 1) * P],
                            x8[:, ci, 2 * k2:2 * k2 + 2, off:off + IB],
                            start=(k2 == 0), stop=(k2 == KD2 - 1 and not KODD),
                            perf_mode=DR,
                        )
                    if KODD:
                        nc.tensor.matmul(
                            pu[:], w8u[:, DT - 1, h * P:(h + 1) * P],
                            x8[:, ci, DT - 1, off:off + IB],
                            start=False, stop=True,
                        )
                    nc.scalar.activation(
                        ut[h][:, ib * IB:(ib + 1) * IB], pu[:], RELU, scale=SCL,
                        accum_out=csu_cols[h][:, ib:ib + 1],
                    )

            csu = [accp.tile([P, 1], f32, tag=f"csu{h}", name=f"csu{h}") for h in range(2)]
            for h in range(2):
                nc.vector.reduce_sum(csu[h][:], csu_cols[h][:], axis=mybir.AxisListType.X)
            bin_c = dram.tile([2, P], f32)
            bout_c = dram.tile([2, P], f32)
            for h in range(2):
                nc.scalar.dma_start(
                    out=bin_c[h, 0:P].rearrange("(p one) -> p one", one=1),
                    in_=csu[h][:],
                )
            nc.gpsimd.collective_compute(
                "AllReduce", mybir.AluOpType.add,
                replica_groups=[list(range(NCORES))],
                ins=[bin_c.opt()], outs=[bout_c.opt()],
            )
            # Gated copy of the T-weights (gate==1.0 exactly, from csu): the
            # deferred T matmuls read wt2, pinning them after C's launch so
            # they fill C's flight time instead of being hoisted earlier.
            gate = accp.tile([P, 1], f32, tag="gate")
            nc.vector.tensor_scalar(
                out=gate[:], in0=csu[0][:], scalar1=0.0, scalar2=1.0,
                op0=mybir.AluOpType.mult, op1=mybir.AluOpType.add,
            )
            wt2 = wp.tile([P, DT, K], bf16, tag="wt2")
            nc.vector.tensor_scalar_mul(wt2[:], wt[:], gate[:])

            # ---- T-passes (xb streamed per block; last TDEF read wt2) ----
            for ib in range(NB):
                xbt = xbts.pop(ib) if ib in xbts else load_xb(ib)
                t_pass(ib, xbt, wt if ib < NB - TDEF else wt2)

            # vtzr = V^T Z (unscaled, bf16) — needs only AllReduce A. Built
            # after the T loop so the DVE queue never head-of-line blocks on
            # A's completion while T drains are pending.
            vtzf = [accp.tile([P, K + 1], f32, tag=f"vtzf{h}", name=f"vtzf{h}") for h in range(2)]
            for h in range(2):
                nc.scalar.dma_start(out=vtzf[h][:], in_=bout_a[h * P:(h + 1) * P, :])
            vtzr = [accp.tile([P, K], bf16, tag=f"vtzr{h}", name=f"vtzr{h}") for h in range(2)]
            for h in range(2):
                nc.vector.tensor_scalar_mul(vtzr[h][:], vtzf[h][:, 0:K], SCL)

            # ---- phase 3: D = 1/(csU.csV/n + eps) as a drain-time scale ----
            csut = accp.tile([P, 2], f32, tag="csut")
            nc.scalar.dma_start(out=csut[:], in_=bout_c.rearrange("t p -> p t"))
            csvt = accp.tile([P, 2], f32, tag="csvt")
            for h in range(2):
                nc.vector.tensor_copy(csvt[:, h:h + 1], vtzf[h][:, K:K + 1])
            pdot = ps.tile([1, 1], f32, tag="work")
            for h in range(2):
                nc.tensor.matmul(
                    pdot[:], csut[:, h:h + 1], csvt[:, h:h + 1],
                    start=(h == 0), stop=(h == 1),
                )
            dsb = accp.tile([1, 1], f32, tag="dsb")
            nc.vector.tensor_scalar(
                out=dsb[:], in0=pdot[:], scalar1=1.0 / (S_V * N_ROWS), scalar2=EPS,
                op0=mybir.AluOpType.mult, op1=mybir.AluOpType.add,
            )
            nc.vector.reciprocal(dsb[:], dsb[:])
            pb = ps.tile([P, 1], f32, tag="work")
            nc.tensor.matmul(pb[:], ones_row[:], dsb[:], start=True, stop=True)
            dbc = accp.tile([P, 1], f32, tag="dbc")
            nc.vector.tensor_copy(dbc[:], pb[:])

            # ---- phase 4: res = (U @ VtZ) * D, batched row-natural writes ----
            # h-major over groups of 4 PSUM tiles: the moving operand stays
            # fixed for the group and each start/stop pair is spread apart,
            # keeping the weight path warm. D lands at the drains (AP scale).
            GG = 4
            for gb in range(NLOC // P // GG):
                prs = [ps.tile([P, K], f32, tag="work", name=f"pr{t}") for t in range(GG)]
                for h in range(2):
                    for t in range(GG):
                        i0 = (gb * GG + t) * P
                        nc.tensor.matmul(
                            prs[t][:], ut[h][:, i0:i0 + P], vtzr[h][:],
                            start=(h == 0), stop=(h == 1),
                        )
                orb = ob.tile([P, GG, K], f32, tag="ob")
                for t in range(GG):
                    # split PSUM->SBUF scaled copies across DVE and ACT
                    if t % 2 == 0:
                        nc.vector.tensor_scalar_mul(orb[:, t, :], prs[t][:], dbc[:])
                    else:
                        nc.scalar.mul(orb[:, t, :], prs[t][:], dbc[:])
                i0 = gb * GG * P
                oq = nc.sync if gb % 2 == 0 else nc.scalar
                oq.dma_start(
                    out=out[i0:i0 + GG * P, 0:K].rearrange(
                        "(s p) c -> p s c", p=P),
                    in_=orb[:],
                )

    nc.compile()
    return nc


def _get_nc(d_rows):
    if d_rows not in _built:
        _built[d_rows] = _build(d_rows)
    return _built[d_rows]


def _q8(a, s):
    return np.clip(a * s, -240.0, 240.0).astype(E4)


def _run(x, W, b, trace=False, trace_cores=None):
    from concourse.bass_utils import run_bass_kernel_spmd

    x = np.ascontiguousarray(x, dtype=np.float32)
    W = np.ascontiguousarray(W, dtype=np.float32)
    b = np.asarray(b, dtype=np.float32)
    if np.any(b):
        d_rows = 1152  # pad contraction: extra ones-row in x picks up b from W
        WT_full = np.zeros((d_rows, 4 * K), np.float32)
        WT_full[:D_IN] = W.T
        WT_full[D_IN] = b
    else:
        d_rows = D_IN
        WT_full = np.ascontiguousarray(W.T)
    DT = d_rows // P
    w8u = np.ascontiguousarray(
        _q8(WT_full[:, 0:K], S_W).reshape(DT, P, K).transpose(1, 0, 2))
    w8vz = np.ascontiguousarray(
        _q8(WT_full[:, K:3 * K], S_W).reshape(DT, P, 2 * K).transpose(1, 0, 2))
    wtt = np.ascontiguousarray(
        WT_full[:, 3 * K:].astype(BF16).reshape(DT, P, K).transpose(1, 0, 2))
    nc = _get_nc(d_rows)
    in_maps = []
    for c in range(NCORES):
        xs = x[c * NLOC:(c + 1) * NLOC]
        if d_rows == D_IN:
            xTs = np.ascontiguousarray(xs.T)
        else:
            xTs = np.zeros((d_rows, NLOC), np.float32)
            xTs[:D_IN] = xs.T
            xTs[D_IN] = 1.0
        xb_bf = xTs.astype(BF16)
        x8f = _q8(xb_bf.astype(np.float32), S_X)
        # pack into block-contiguous layouts so every device load is one
        # [128 x 8KB] 2D DMA (the row-gather pattern runs at ~83GB/s)
        xb_p = np.ascontiguousarray(
            xb_bf.reshape(DT, P, NB, IB).transpose(2, 1, 0, 3))
        x8_p = np.ascontiguousarray(
            x8f.reshape(DT, P, NLOC // 1024, 1024).transpose(2, 1, 0, 3))
        in_maps.append({"x8": x8_p, "xb": xb_p, "w8u": w8u, "w8vz": w8vz, "wtt": wtt})
    res = run_bass_kernel_spmd(
        nc, in_maps, list(range(NCORES)),
        trace=trace, **({"trace_cores": trace_cores} if trace_cores else {}),
    )
    full = np.concatenate([res.results[c]["out"] for c in range(NCORES)], axis=0)
    return full, res


def kernel(x, W, b):
    full, _ = _run(x, W, b)
    return full


# revision 14
# speedup vs baseline: 1.3010x; 1.0340x over previous
"""Low-rank attention Trainium2 kernel (8 NeuronCores, SPMD), fp8 edition.

Math (reference):
    tmp = relu(x @ W.T + b); U,V,Z,T = split(tmp, 4, axis=1)
    norm = sum(U @ colsum(V)) / n + eps ;  D = 1/norm
    out = concat[(U @ (V.T @ Z)) * D, T]

Sharding: rows of x across 8 cores. Per-core partials (V.T@Z [k,k],
colsum(V), colsum(U)) are AllReduced on-device; each core then computes
its local U @ (VtZ) * D.

fp8 design (vs the 339us bf16 baseline):
- U, V, Z projections and V^T@Z run as fp8e4 DoubleRow matmuls (2 k-tiles
  per instruction, measured 216ns steady for moving-512 = true 2x bf16;
  LDWEIGHTS hides behind the previous matmul's streaming).
- x is quantized to fp8 on the HOST (x8 = e4m3(16x), 8MB/core, resident);
  on-device bf16->fp8 converts are not viable (only DVE writes fp8 fast).
- The T block stays bf16 (its error hits the output directly; fp8's ~2.5%
  elementwise would eat the whole 2e-2 budget). bf16 x streams through a
  rolling pool, one [1024, 512] block per T-pass block.
- Scales: x8 = 16x, W8 = 64W, vz fp8 = 32*[V|Z]; U drains unscaled (bf16,
  ACT relu scale 1/1024 + csu accum); VtZ psum = 1024 V^T Z | 32 csV.

Collective hiding (the bf16 baseline exposed ~41us of AllReduce):
- Phase 1a: V|Z + V^T@Z for ALL i-blocks first; V^T@Z accumulates across
  blocks in two PSUM chains (no per-block DVE adds). AllReduce A (the
  whole [k,k+1] x 2 payload) launches at ~1/3 of the kernel.
- Phase 1b: all U-passes; then AllReduce C (csu only, 1KB).
- T-passes follow (last TDEF read a csu-gated copy of the T-weights,
  pinning them after C's launch); phase-4 matmuls need only A's result
  (vtzr = V^T Z unscaled); the data-dependent D = 1/norm is applied at
  the phase-4 PSUM drains as a per-partition AP scale, so C's latency
  hides under the T-pass + phase-4 matmuls.
- DMA rings: x8 + T-out on sync, weights + staging + res-out on scalar,
  xb blocks + collectives on gpsimd.
"""
import sys

sys.path.insert(0, "/opt/trn_rl_repo")
import numpy as np
import ml_dtypes

BF16 = ml_dtypes.bfloat16
E4 = ml_dtypes.float8_e4m3

NCORES = 8
N_ROWS, D_IN, K = 65536, 1024, 256
NLOC = N_ROWS // NCORES      # 8192 rows per core
P = 128
IB = 512                     # i-block width
NB = NLOC // IB              # 16 blocks
EPS = 1e-6
TDEF = 6                     # T-pass blocks deferred behind AllReduce C
S_X, S_W, S_V = 16.0, 64.0, 32.0
X8CHUNKS = [(0, 512), (512, 512), (1024, 1024), (2048, 2048), (4096, 4096)]

_built = {}


def _build(d_rows):
    import concourse.bacc as bacc
    import concourse.mybir as mybir
    import concourse.tile as tile

    dt = mybir.dt
    f32, bf16, f8 = dt.float32, dt.bfloat16, dt.float8e4
    RELU = mybir.ActivationFunctionType.Relu
    DR = mybir.MatmulPerfMode.DoubleRow
    DT = d_rows // P
    KD2 = DT // 2            # DoubleRow kd-pairs
    KODD = DT % 2            # leftover plain-fp8 k-tile (bias-pad path)
    NSUB = IB // P
    SCL = 1.0 / (S_X * S_W)  # psum -> true pre-activation

    nc = bacc.Bacc("TRN2", target_bir_lowering=False, debug=False, num_devices=NCORES)
    NBX = NLOC // 1024       # x8 resident-load chunks
    x8d = nc.dram_tensor("x8", [NBX, P, DT, 1024], f8, kind="ExternalInput")
    xbd = nc.dram_tensor("xb", [NB, P, DT, IB], bf16, kind="ExternalInput")
    w8ud = nc.dram_tensor("w8u", [P, DT, K], f8, kind="ExternalInput")
    w8vzd = nc.dram_tensor("w8vz", [P, DT, 2 * K], f8, kind="ExternalInput")
    wttd = nc.dram_tensor("wtt", [P, DT, K], bf16, kind="ExternalInput")
    out = nc.dram_tensor("out", [NLOC, 2 * K], f32, kind="ExternalOutput")

    with tile.TileContext(nc) as tc:
        with (
            tc.tile_pool(name="wp", bufs=1) as wp,
            tc.tile_pool(name="xp", bufs=1) as xp,
            tc.tile_pool(name="xbp", bufs=7) as xbp,
            tc.tile_pool(name="up", bufs=1) as up,
            tc.tile_pool(name="vzp", bufs=6) as vzp,
            tc.tile_pool(name="ob", bufs=3) as ob,
            tc.tile_pool(name="acc", bufs=1) as accp,
            tc.tile_pool(name="ps", bufs=6, space="PSUM") as ps,
            tc.tile_pool(name="ps2", bufs=1, space="PSUM") as ps2,
            tc.tile_pool(name="dram", bufs=1, space="DRAM") as dram,
        ):
            # Weights (gpsimd/scalar rings), then resident x8 in per-kd
            # column chunks (sync ring, small first chunks so ib0 starts
            # early). bf16 x streams per-block via xbp below.
            w8vz = wp.tile([P, DT, 2 * K], f8, tag="w8vz")
            nc.scalar.dma_start(out=w8vz[:], in_=w8vzd[:])
            w8u = wp.tile([P, DT, K], f8, tag="w8u")
            nc.scalar.dma_start(out=w8u[:], in_=w8ud[:])
            ut = [up.tile([P, NLOC], bf16, tag=f"ut{h}", name=f"ut{h}") for h in range(2)]
            xbts = {}

            def load_xb(ib):
                xbt = xbp.tile([P, DT, IB], bf16, tag="xb", name=f"xb{ib}")
                q = nc.sync if ib % 2 == 0 else nc.scalar
                q.dma_start(out=xbt[:], in_=xbd[ib])
                return xbt

            XC = 1024
            x8 = xp.tile([P, NBX, DT, XC], f8, tag="x8")
            for ci in range(NBX):
                q = nc.sync if ci % 2 == 0 else nc.scalar
                q.dma_start(out=x8[:, ci, :, :], in_=x8d[ci])
                if ci == 1:
                    wt = wp.tile([P, DT, K], bf16, tag="wt")
                    nc.scalar.dma_start(out=wt[:], in_=wttd[:])
                if 1 <= ci <= 7:
                    xbts[ci - 1] = load_xb(ci - 1)
            ones_row = wp.tile([1, P], f32, tag="ones_row")
            nc.vector.memset(ones_row[:], 1.0)

            csu_cols = [accp.tile([P, NB], f32, tag=f"csuc{h}", name=f"csuc{h}") for h in range(2)]

            def t_pass(ib, xbt, wsrc):
                """T = relu(x @ Wt): 4 row-subtiles, one batched out-DMA."""
                otb = ob.tile([P, NSUB, K], f32, tag="ob")
                for s in range(NSUB):
                    pt = ps.tile([P, K], f32, tag="work")
                    for kd in range(DT):
                        nc.tensor.matmul(
                            pt[:], xbt[:, kd, s * P:(s + 1) * P],
                            wsrc[:, kd, :],
                            start=(kd == 0), stop=(kd == DT - 1),
                        )
                    nc.vector.tensor_relu(otb[:, s, :], pt[:])
                i0 = ib * IB
                nc.sync.dma_start(
                    out=out[i0:i0 + IB, K:2 * K].rearrange(
                        "(s p) c -> p s c", p=P),
                    in_=otb[:],
                )

            # ---- phase 1a: V|Z fp8 projection + V^T@Z PSUM chains ----
            # vz col 512 = 1.0 rides the V^T@Z matmul to produce 32*csV in
            # column 256 of the [k, k+1] chain.
            pzh = [ps2.tile([P, K + 1], f32, tag=f"pz{h}", name=f"pz{h}") for h in range(2)]

            def vtz(ib, vz_tiles):
                for h in range(2):
                    for sp in range(NSUB // 2):
                        nc.tensor.matmul(
                            pzh[h][:], vz_tiles[sp][:, :, h * P:(h + 1) * P],
                            vz_tiles[sp][:, :, K:2 * K + 1],
                            start=(ib == 0 and sp == 0),
                            stop=(ib == NB - 1 and sp == NSUB // 2 - 1),
                            perf_mode=DR,
                        )

            prev_vz = None
            for ib in range(NB):
                vz_tiles = []
                for sp in range(NSUB // 2):
                    vzt = vzp.tile([P, 2, 2 * K + 16], f8, tag="vz")
                    for s2 in range(2):
                        s = sp * 2 + s2
                        pvz = ps.tile([P, IB], f32, tag="work")
                        ci, off = ib // 2, (ib % 2) * IB + s * P
                        for k2 in range(KD2):
                            nc.tensor.matmul(
                                pvz[:],
                                x8[:, ci, 2 * k2:2 * k2 + 2, off:off + P],
                                w8vz[:, 2 * k2:2 * k2 + 2, :],
                                start=(k2 == 0), stop=(k2 == KD2 - 1 and not KODD),
                                perf_mode=DR,
                            )
                        if KODD:
                            nc.tensor.matmul(
                                pvz[:],
                                x8[:, ci, DT - 1, off:off + P],
                                w8vz[:, DT - 1, :],
                                start=False, stop=True,
                            )
                        nc.vector.tensor_scalar(
                            out=vzt[:, s2, 0:2 * K], in0=pvz[:],
                            scalar1=S_V * SCL, scalar2=0.0,
                            op0=mybir.AluOpType.mult, op1=mybir.AluOpType.max,
                        )
                    nc.vector.memset(vzt[:, :, 2 * K:2 * K + 1], 1.0)
                    vz_tiles.append(vzt)
                if prev_vz is not None:
                    vtz(ib - 1, prev_vz)
                prev_vz = vz_tiles
            vtz(NB - 1, prev_vz)

            # ---- AllReduce A: the full V^T@Z | csV payload ----
            bin_a = dram.tile([2 * P, K + 1], f32)
            bout_a = dram.tile([2 * P, K + 1], f32)
            vtzs = [accp.tile([P, K + 1], f32, tag=f"vtzs{h}", name=f"vtzs{h}") for h in range(2)]
            for h in range(2):
                nc.vector.tensor_copy(vtzs[h][:], pzh[h][:])
                nc.scalar.dma_start(out=bin_a[h * P:(h + 1) * P, :], in_=vtzs[h][:])
            nc.gpsimd.collective_compute(
                "AllReduce", mybir.AluOpType.add,
                replica_groups=[list(range(NCORES))],
                ins=[bin_a.opt()], outs=[bout_a.opt()],
            )

            # ---- phase 1b: all U-passes (fp8 DR), then AllReduce C (csu) ----
            for ib in range(NB):
                for h in range(2):
                    pu = ps.tile([P, IB], f32, tag="work")
                    ci, off = ib // 2, (ib % 2) * IB
                    for k2 in range(KD2):
                        nc.tensor.matmul(
                            pu[:], w8u[:, 2 * k2:2 * k2 + 2, h * P:(h + 1) * P],
                            x8[:, ci, 2 * k2:2 * k2 + 2, off:off + IB],
                            start=(k2 == 0), stop=(k2 == KD2 - 1 and not KODD),
                            perf_mode=DR,
                        )
                    if KODD:
                        nc.tensor.matmul(
                            pu[:], w8u[:, DT - 1, h * P:(h + 1) * P],
                            x8[:, ci, DT - 1, off:off + IB],
                            start=False, stop=True,
                        )
                    nc.scalar.activation(
                        ut[h][:, ib * IB:(ib + 1) * IB], pu[:], RELU, scale=SCL,
                        accum_out=csu_cols[h][:, ib:ib + 1],
                    )

            csu = [accp.tile([P, 1], f32, tag=f"csu{h}", name=f"csu{h}") for h in range(2)]
            for h in range(2):
                nc.vector.reduce_sum(csu[h][:], csu_cols[h][:], axis=mybir.AxisListType.X)
            bin_c = dram.tile([2, P], f32)
            bout_c = dram.tile([2, P], f32)
            for h in range(2):
                nc.scalar.dma_start(
                    out=bin_c[h, 0:P].rearrange("(p one) -> p one", one=1),
                    in_=csu[h][:],
                )
            nc.gpsimd.collective_compute(
                "AllReduce", mybir.AluOpType.add,
                replica_groups=[list(range(NCORES))],
                ins=[bin_c.opt()], outs=[bout_c.opt()],
            )
            # Gated copy of the T-weights (gate==1.0 exactly, from csu): the
            # deferred T matmuls read wt2, pinning them after C's launch so
            # they fill C's flight time instead of being hoisted earlier.
            gate = accp.tile([P, 1], f32, tag="gate")
            nc.vector.tensor_scalar(
                out=gate[:], in0=csu[0][:], scalar1=0.0, scalar2=1.0,
                op0=mybir.AluOpType.mult, op1=mybir.AluOpType.add,
            )
            wt2 = wp.tile([P, DT, K], bf16, tag="wt2")
            nc.vector.tensor_scalar_mul(wt2[:], wt[:], gate[:])

            # ---- T-passes (xb streamed per block; last TDEF read wt2) ----
            for ib in range(NB):
                xbt = xbts.pop(ib) if ib in xbts else load_xb(ib)
                t_pass(ib, xbt, wt if ib < NB - TDEF else wt2)

            # vtzr = V^T Z (unscaled, bf16) — needs only AllReduce A. Built
            # after the T loop so the DVE queue never head-of-line blocks on
            # A's completion while T drains are pending.
            vtzf = [accp.tile([P, K + 1], f32, tag=f"vtzf{h}", name=f"vtzf{h}") for h in range(2)]
            for h in range(2):
                nc.scalar.dma_start(out=vtzf[h][:], in_=bout_a[h * P:(h + 1) * P, :])
            vtzr = [accp.tile([P, K], bf16, tag=f"vtzr{h}", name=f"vtzr{h}") for h in range(2)]
            for h in range(2):
                nc.vector.tensor_scalar_mul(vtzr[h][:], vtzf[h][:, 0:K], SCL)

            # ---- phase 3: D = 1/(csU.csV/n + eps) as a drain-time scale ----
            csut = accp.tile([P, 2], f32, tag="csut")
            nc.scalar.dma_start(out=csut[:], in_=bout_c.rearrange("t p -> p t"))
            csvt = accp.tile([P, 2], f32, tag="csvt")
            for h in range(2):
                nc.vector.tensor_copy(csvt[:, h:h + 1], vtzf[h][:, K:K + 1])
            pdot = ps.tile([1, 1], f32, tag="work")
            for h in range(2):
                nc.tensor.matmul(
                    pdot[:], csut[:, h:h + 1], csvt[:, h:h + 1],
                    start=(h == 0), stop=(h == 1),
                )
            dsb = accp.tile([1, 1], f32, tag="dsb")
            nc.vector.tensor_scalar(
                out=dsb[:], in0=pdot[:], scalar1=1.0 / (S_V * N_ROWS), scalar2=EPS,
                op0=mybir.AluOpType.mult, op1=mybir.AluOpType.add,
            )
            nc.vector.reciprocal(dsb[:], dsb[:])
            pb = ps.tile([P, 1], f32, tag="work")
            nc.tensor.matmul(pb[:], ones_row[:], dsb[:], start=True, stop=True)
            dbc = accp.tile([P, 1], f32, tag="dbc")
            nc.vector.tensor_copy(dbc[:], pb[:])

            # ---- phase 4: res = (U @ VtZ) * D, batched row-natural writes ----
            # h-major over groups of 4 PSUM tiles: the moving operand stays
            # fixed for the group and each start/stop pair is spread apart,
            # keeping the weight path warm. D lands at the drains (AP scale).
            GG = 4
            for gb in range(NLOC // P // GG):
                prs = [ps.tile([P, K], f32, tag="work", name=f"pr{t}") for t in range(GG)]
                for h in range(2):
                    for t in range(GG):
                        i0 = (gb * GG + t) * P
                        nc.tensor.matmul(
                            prs[t][:], ut[h][:, i0:i0 + P], vtzr[h][:],
                            start=(h == 0), stop=(h == 1),
                        )
                orb = ob.tile([P, GG, K], f32, tag="ob")
                for t in range(GG):
                    # split PSUM->SBUF scaled copies across DVE and ACT
                    if t % 2 == 0:
                        nc.vector.tensor_scalar_mul(orb[:, t, :], prs[t][:], dbc[:])
                    else:
                        nc.scalar.mul(orb[:, t, :], prs[t][:], dbc[:])
                i0 = gb * GG * P
                oq = nc.sync if gb % 2 == 0 else nc.scalar
                oq.dma_start(
                    out=out[i0:i0 + GG * P, 0:K].rearrange(
                        "(s p) c -> p s c", p=P),
                    in_=orb[:],
                )

    nc.compile()
    return nc


def _get_nc(d_rows):
    if d_rows not in _built:
        _built[d_rows] = _build(d_rows)
    return _built[d_rows]


def _q8(a, s):
    return np.clip(a * s, -240.0, 240.0).astype(E4)


def _run(x, W, b, trace=False, trace_cores=None):
    from concourse.bass_utils import run_bass_kernel_spmd

    x = np.ascontiguousarray(x, dtype=np.float32)
    W = np.ascontiguousarray(W, dtype=np.float32)
    b = np.asarray(b, dtype=np.float32)
    if np.any(b):
        d_rows = 1152  # pad contraction: extra ones-row in x picks up b from W
        WT_full = np.zeros((d_rows, 4 * K), np.float32)
        WT_full[:D_IN] = W.T
        WT_full[D_IN] = b
    else:
        d_rows = D_IN
        WT_full = np.ascontiguousarray(W.T)
    DT = d_rows // P
    w8u = np.ascontiguousarray(
        _q8(WT_full[:, 0:K], S_W).reshape(DT, P, K).transpose(1, 0, 2))
    w8vz = np.ascontiguousarray(
        _q8(WT_full[:, K:3 * K], S_W).reshape(DT, P, 2 * K).transpose(1, 0, 2))
    wtt = np.ascontiguousarray(
        WT_full[:, 3 * K:].astype(BF16).reshape(DT, P, K).transpose(1, 0, 2))
    nc = _get_nc(d_rows)
    in_maps = []
    for c in range(NCORES):
        xs = x[c * NLOC:(c + 1) * NLOC]
        if d_rows == D_IN:
            xTs = np.ascontiguousarray(xs.T)
        else:
            xTs = np.zeros((d_rows, NLOC), np.float32)
            xTs[:D_IN] = xs.T
            xTs[D_IN] = 1.0
        xb_bf = xTs.astype(BF16)
        x8f = _q8(xb_bf.astype(np.float32), S_X)
        # pack into block-contiguous layouts so every device load is one
        # [128 x 8KB] 2D DMA (the row-gather pattern runs at ~83GB/s)
        xb_p = np.ascontiguousarray(
            xb_bf.reshape(DT, P, NB, IB).transpose(2, 1, 0, 3))
        x8_p = np.ascontiguousarray(
            x8f.reshape(DT, P, NLOC // 1024, 1024).transpose(2, 1, 0, 3))
        in_maps.append({"x8": x8_p, "xb": xb_p, "w8u": w8u, "w8vz": w8vz, "wtt": wtt})
    res = run_bass_kernel_spmd(
        nc, in_maps, list(range(NCORES)),
        trace=trace, **({"trace_cores": trace_cores} if trace_cores else {}),
    )
    full = np.concatenate([res.results[c]["out"] for c in range(NCORES)], axis=0)
    return full, res


def kernel(x, W, b):
    full, _ = _run(x, W, b)
    return full


# revision 15
# speedup vs baseline: 1.3211x; 1.0154x over previous
"""Low-rank attention Trainium2 kernel (8 NeuronCores, SPMD), fp8 edition.

Math (reference):
    tmp = relu(x @ W.T + b); U,V,Z,T = split(tmp, 4, axis=1)
    norm = sum(U @ colsum(V)) / n + eps ;  D = 1/norm
    out = concat[(U @ (V.T @ Z)) * D, T]

Sharding: rows of x across 8 cores. Per-core partials (V.T@Z [k,k],
colsum(V), colsum(U)) are AllReduced on-device; each core then computes
its local U @ (VtZ) * D.

fp8 design (vs the 339us bf16 baseline):
- U, V, Z projections and V^T@Z run as fp8e4 DoubleRow matmuls (2 k-tiles
  per instruction, measured 216ns steady for moving-512 = true 2x bf16;
  LDWEIGHTS hides behind the previous matmul's streaming).
- x is quantized to fp8 on the HOST (x8 = e4m3(16x), 8MB/core, resident);
  on-device bf16->fp8 converts are not viable (only DVE writes fp8 fast).
- The T block stays bf16 (its error hits the output directly; fp8's ~2.5%
  elementwise would eat the whole 2e-2 budget). bf16 x streams through a
  rolling pool, one [1024, 512] block per T-pass block.
- Scales: x8 = 16x, W8 = 64W, vz fp8 = 32*[V|Z]; U drains unscaled (bf16,
  ACT relu scale 1/1024 + csu accum); VtZ psum = 1024 V^T Z | 32 csV.

Collective hiding (the bf16 baseline exposed ~41us of AllReduce):
- Phase 1a: V|Z + V^T@Z for ALL i-blocks first; V^T@Z accumulates across
  blocks in two PSUM chains (no per-block DVE adds). AllReduce A (the
  whole [k,k+1] x 2 payload) launches at ~1/3 of the kernel.
- Phase 1b: all U-passes; then AllReduce C (csu only, 1KB).
- T-passes follow (last TDEF read a csu-gated copy of the T-weights,
  pinning them after C's launch); phase-4 matmuls need only A's result
  (vtzr = V^T Z unscaled); the data-dependent D = 1/norm is applied at
  the phase-4 PSUM drains as a per-partition AP scale, so C's latency
  hides under the T-pass + phase-4 matmuls.
- DMA rings: x8 + T-out on sync, weights + staging + res-out on scalar,
  xb blocks + collectives on gpsimd.
"""
import sys

sys.path.insert(0, "/opt/trn_rl_repo")
import numpy as np
import ml_dtypes

BF16 = ml_dtypes.bfloat16
E4 = ml_dtypes.float8_e4m3

NCORES = 8
N_ROWS, D_IN, K = 65536, 1024, 256
NLOC = N_ROWS // NCORES      # 8192 rows per core
P = 128
IB = 512                     # i-block width
NB = NLOC // IB              # 16 blocks
EPS = 1e-6
TDEF = 6                     # T-pass blocks deferred behind AllReduce C
S_X, S_W, S_V = 16.0, 64.0, 32.0
X8CHUNKS = [(0, 512), (512, 512), (1024, 1024), (2048, 2048), (4096, 4096)]

_built = {}


def _build(d_rows):
    import concourse.bacc as bacc
    import concourse.mybir as mybir
    import concourse.tile as tile

    dt = mybir.dt
    f32, bf16, f8 = dt.float32, dt.bfloat16, dt.float8e4
    RELU = mybir.ActivationFunctionType.Relu
    DR = mybir.MatmulPerfMode.DoubleRow
    DT = d_rows // P
    KD2 = DT // 2            # DoubleRow kd-pairs
    KODD = DT % 2            # leftover plain-fp8 k-tile (bias-pad path)
    NSUB = IB // P
    SCL = 1.0 / (S_X * S_W)  # psum -> true pre-activation

    nc = bacc.Bacc("TRN2", target_bir_lowering=False, debug=False, num_devices=NCORES)
    NBX = NLOC // 1024       # x8 resident-load chunks
    x8d = nc.dram_tensor("x8", [NBX, P, DT, 1024], f8, kind="ExternalInput")
    xbd = nc.dram_tensor("xb", [NB, P, DT, IB], bf16, kind="ExternalInput")
    w8ud = nc.dram_tensor("w8u", [P, DT, K], f8, kind="ExternalInput")
    w8vzd = nc.dram_tensor("w8vz", [P, DT, 2 * K], f8, kind="ExternalInput")
    wttd = nc.dram_tensor("wtt", [P, DT, K], bf16, kind="ExternalInput")
    out = nc.dram_tensor("out", [NLOC, 2 * K], f32, kind="ExternalOutput")

    with tile.TileContext(nc) as tc:
        with (
            tc.tile_pool(name="wp", bufs=1) as wp,
            tc.tile_pool(name="xp", bufs=1) as xp,
            tc.tile_pool(name="xbp", bufs=7) as xbp,
            tc.tile_pool(name="up", bufs=1) as up,
            tc.tile_pool(name="vzp", bufs=6) as vzp,
            tc.tile_pool(name="ob", bufs=3) as ob,
            tc.tile_pool(name="acc", bufs=1) as accp,
            tc.tile_pool(name="ps", bufs=6, space="PSUM") as ps,
            tc.tile_pool(name="ps2", bufs=1, space="PSUM") as ps2,
            tc.tile_pool(name="dram", bufs=1, space="DRAM") as dram,
        ):
            # Weights (gpsimd/scalar rings), then resident x8 in per-kd
            # column chunks (sync ring, small first chunks so ib0 starts
            # early). bf16 x streams per-block via xbp below.
            w8vz = wp.tile([P, DT, 2 * K], f8, tag="w8vz")
            nc.scalar.dma_start(out=w8vz[:], in_=w8vzd[:])
            w8u = wp.tile([P, DT, K], f8, tag="w8u")
            nc.scalar.dma_start(out=w8u[:], in_=w8ud[:])
            ut = [up.tile([P, NLOC], bf16, tag=f"ut{h}", name=f"ut{h}") for h in range(2)]
            xbts = {}

            def load_xb(ib):
                xbt = xbp.tile([P, DT, IB], bf16, tag="xb", name=f"xb{ib}")
                nc.scalar.dma_start(out=xbt[:], in_=xbd[ib])
                return xbt

            XC = 1024
            x8 = xp.tile([P, NBX, DT, XC], f8, tag="x8")
            for ci in range(NBX):
                q = nc.sync if ci % 2 == 0 else nc.scalar
                q.dma_start(out=x8[:, ci, :, :], in_=x8d[ci])
                if ci == 1:
                    wt = wp.tile([P, DT, K], bf16, tag="wt")
                    nc.scalar.dma_start(out=wt[:], in_=wttd[:])
                if 1 <= ci <= 7:
                    xbts[ci - 1] = load_xb(ci - 1)
            ones_row = wp.tile([1, P], f32, tag="ones_row")
            nc.vector.memset(ones_row[:], 1.0)

            csu_cols = [accp.tile([P, NB], f32, tag=f"csuc{h}", name=f"csuc{h}") for h in range(2)]

            def t_pass(ib, xbt, wsrc):
                """T = relu(x @ Wt): 4 row-subtiles, one batched out-DMA."""
                otb = ob.tile([P, NSUB, K], f32, tag="ob")
                for s in range(NSUB):
                    pt = ps.tile([P, K], f32, tag="work")
                    for kd in range(DT):
                        nc.tensor.matmul(
                            pt[:], xbt[:, kd, s * P:(s + 1) * P],
                            wsrc[:, kd, :],
                            start=(kd == 0), stop=(kd == DT - 1),
                        )
                    nc.vector.tensor_relu(otb[:, s, :], pt[:])
                i0 = ib * IB
                nc.sync.dma_start(
                    out=out[i0:i0 + IB, K:2 * K].rearrange(
                        "(s p) c -> p s c", p=P),
                    in_=otb[:],
                )

            # ---- phase 1a: V|Z fp8 projection + V^T@Z PSUM chains ----
            # vz col 512 = 1.0 rides the V^T@Z matmul to produce 32*csV in
            # column 256 of the [k, k+1] chain.
            pzh = [ps2.tile([P, K + 1], f32, tag=f"pz{h}", name=f"pz{h}") for h in range(2)]

            def vtz(ib, vz_tiles):
                for h in range(2):
                    for sp in range(NSUB // 2):
                        nc.tensor.matmul(
                            pzh[h][:], vz_tiles[sp][:, :, h * P:(h + 1) * P],
                            vz_tiles[sp][:, :, K:2 * K + 1],
                            start=(ib == 0 and sp == 0),
                            stop=(ib == NB - 1 and sp == NSUB // 2 - 1),
                            perf_mode=DR,
                        )

            prev_vz = None
            for ib in range(NB):
                vz_tiles = []
                for sp in range(NSUB // 2):
                    vzt = vzp.tile([P, 2, 2 * K + 16], f8, tag="vz")
                    for s2 in range(2):
                        s = sp * 2 + s2
                        pvz = ps.tile([P, IB], f32, tag="work")
                        ci, off = ib // 2, (ib % 2) * IB + s * P
                        for k2 in range(KD2):
                            nc.tensor.matmul(
                                pvz[:],
                                x8[:, ci, 2 * k2:2 * k2 + 2, off:off + P],
                                w8vz[:, 2 * k2:2 * k2 + 2, :],
                                start=(k2 == 0), stop=(k2 == KD2 - 1 and not KODD),
                                perf_mode=DR,
                            )
                        if KODD:
                            nc.tensor.matmul(
                                pvz[:],
                                x8[:, ci, DT - 1, off:off + P],
                                w8vz[:, DT - 1, :],
                                start=False, stop=True,
                            )
                        nc.vector.tensor_scalar(
                            out=vzt[:, s2, 0:2 * K], in0=pvz[:],
                            scalar1=S_V * SCL, scalar2=0.0,
                            op0=mybir.AluOpType.mult, op1=mybir.AluOpType.max,
                        )
                    nc.vector.memset(vzt[:, :, 2 * K:2 * K + 1], 1.0)
                    vz_tiles.append(vzt)
                if prev_vz is not None:
                    vtz(ib - 1, prev_vz)
                prev_vz = vz_tiles
            vtz(NB - 1, prev_vz)

            # ---- AllReduce A: the full V^T@Z | csV payload ----
            bin_a = dram.tile([2 * P, K + 1], f32)
            bout_a = dram.tile([2 * P, K + 1], f32)
            vtzs = [accp.tile([P, K + 1], f32, tag=f"vtzs{h}", name=f"vtzs{h}") for h in range(2)]
            for h in range(2):
                nc.vector.tensor_copy(vtzs[h][:], pzh[h][:])
                nc.scalar.dma_start(out=bin_a[h * P:(h + 1) * P, :], in_=vtzs[h][:])
            nc.gpsimd.collective_compute(
                "AllReduce", mybir.AluOpType.add,
                replica_groups=[list(range(NCORES))],
                ins=[bin_a.opt()], outs=[bout_a.opt()],
            )

            # ---- phase 1b: all U-passes (fp8 DR), then AllReduce C (csu) ----
            for ib in range(NB):
                for h in range(2):
                    pu = ps.tile([P, IB], f32, tag="work")
                    ci, off = ib // 2, (ib % 2) * IB
                    for k2 in range(KD2):
                        nc.tensor.matmul(
                            pu[:], w8u[:, 2 * k2:2 * k2 + 2, h * P:(h + 1) * P],
                            x8[:, ci, 2 * k2:2 * k2 + 2, off:off + IB],
                            start=(k2 == 0), stop=(k2 == KD2 - 1 and not KODD),
                            perf_mode=DR,
                        )
                    if KODD:
                        nc.tensor.matmul(
                            pu[:], w8u[:, DT - 1, h * P:(h + 1) * P],
                            x8[:, ci, DT - 1, off:off + IB],
                            start=False, stop=True,
                        )
                    nc.scalar.activation(
                        ut[h][:, ib * IB:(ib + 1) * IB], pu[:], RELU, scale=SCL,
                        accum_out=csu_cols[h][:, ib:ib + 1],
                    )

            csu = [accp.tile([P, 1], f32, tag=f"csu{h}", name=f"csu{h}") for h in range(2)]
            for h in range(2):
                nc.vector.reduce_sum(csu[h][:], csu_cols[h][:], axis=mybir.AxisListType.X)
            bin_c = dram.tile([2, P], f32)
            bout_c = dram.tile([2, P], f32)
            for h in range(2):
                nc.scalar.dma_start(
                    out=bin_c[h, 0:P].rearrange("(p one) -> p one", one=1),
                    in_=csu[h][:],
                )
            nc.gpsimd.collective_compute(
                "AllReduce", mybir.AluOpType.add,
                replica_groups=[list(range(NCORES))],
                ins=[bin_c.opt()], outs=[bout_c.opt()],
            )
            # Gated copy of the T-weights (gate==1.0 exactly, from csu): the
            # deferred T matmuls read wt2, pinning them after C's launch so
            # they fill C's flight time instead of being hoisted earlier.
            gate = accp.tile([P, 1], f32, tag="gate")
            nc.vector.tensor_scalar(
                out=gate[:], in0=csu[0][:], scalar1=0.0, scalar2=1.0,
                op0=mybir.AluOpType.mult, op1=mybir.AluOpType.add,
            )
            wt2 = wp.tile([P, DT, K], bf16, tag="wt2")
            nc.vector.tensor_scalar_mul(wt2[:], wt[:], gate[:])

            # ---- T-passes (xb streamed per block; last TDEF read wt2) ----
            for ib in range(NB):
                xbt = xbts.pop(ib) if ib in xbts else load_xb(ib)
                t_pass(ib, xbt, wt if ib < NB - TDEF else wt2)

            # vtzr = V^T Z (unscaled, bf16) — needs only AllReduce A. Built
            # after the T loop so the DVE queue never head-of-line blocks on
            # A's completion while T drains are pending.
            vtzf = [accp.tile([P, K + 1], f32, tag=f"vtzf{h}", name=f"vtzf{h}") for h in range(2)]
            for h in range(2):
                nc.scalar.dma_start(out=vtzf[h][:], in_=bout_a[h * P:(h + 1) * P, :])
            vtzr = [accp.tile([P, K], bf16, tag=f"vtzr{h}", name=f"vtzr{h}") for h in range(2)]
            for h in range(2):
                nc.vector.tensor_scalar_mul(vtzr[h][:], vtzf[h][:, 0:K], SCL)

            # ---- phase 3: D = 1/(csU.csV/n + eps) as a drain-time scale ----
            csut = accp.tile([P, 2], f32, tag="csut")
            nc.scalar.dma_start(out=csut[:], in_=bout_c.rearrange("t p -> p t"))
            csvt = accp.tile([P, 2], f32, tag="csvt")
            for h in range(2):
                nc.vector.tensor_copy(csvt[:, h:h + 1], vtzf[h][:, K:K + 1])
            pdot = ps.tile([1, 1], f32, tag="work")
            for h in range(2):
                nc.tensor.matmul(
                    pdot[:], csut[:, h:h + 1], csvt[:, h:h + 1],
                    start=(h == 0), stop=(h == 1),
                )
            dsb = accp.tile([1, 1], f32, tag="dsb")
            nc.vector.tensor_scalar(
                out=dsb[:], in0=pdot[:], scalar1=1.0 / (S_V * N_ROWS), scalar2=EPS,
                op0=mybir.AluOpType.mult, op1=mybir.AluOpType.add,
            )
            nc.vector.reciprocal(dsb[:], dsb[:])
            pb = ps.tile([P, 1], f32, tag="work")
            nc.tensor.matmul(pb[:], ones_row[:], dsb[:], start=True, stop=True)
            dbc = accp.tile([P, 1], f32, tag="dbc")
            nc.vector.tensor_copy(dbc[:], pb[:])

            # ---- phase 4: res = (U @ VtZ) * D, batched row-natural writes ----
            # h-major over groups of 4 PSUM tiles: the moving operand stays
            # fixed for the group and each start/stop pair is spread apart,
            # keeping the weight path warm. D lands at the drains (AP scale).
            GG = 4
            for gb in range(NLOC // P // GG):
                prs = [ps.tile([P, K], f32, tag="work", name=f"pr{t}") for t in range(GG)]
                for h in range(2):
                    for t in range(GG):
                        i0 = (gb * GG + t) * P
                        nc.tensor.matmul(
                            prs[t][:], ut[h][:, i0:i0 + P], vtzr[h][:],
                            start=(h == 0), stop=(h == 1),
                        )
                orb = ob.tile([P, GG, K], f32, tag="ob")
                for t in range(GG):
                    # split PSUM->SBUF scaled copies across DVE and ACT
                    if t % 2 == 0:
                        nc.vector.tensor_scalar_mul(orb[:, t, :], prs[t][:], dbc[:])
                    else:
                        nc.scalar.mul(orb[:, t, :], prs[t][:], dbc[:])
                i0 = gb * GG * P
                nc.sync.dma_start(
                    out=out[i0:i0 + GG * P, 0:K].rearrange(
                        "(s p) c -> p s c", p=P),
                    in_=orb[:],
                )

    nc.compile()
    return nc


def _get_nc(d_rows):
    if d_rows not in _built:
        _built[d_rows] = _build(d_rows)
    return _built[d_rows]


def _q8(a, s):
    return np.clip(a * s, -240.0, 240.0).astype(E4)


def _run(x, W, b, trace=False, trace_cores=None):
    from concourse.bass_utils import run_bass_kernel_spmd

    x = np.ascontiguousarray(x, dtype=np.float32)
    W = np.ascontiguousarray(W, dtype=np.float32)
    b = np.asarray(b, dtype=np.float32)
    if np.any(b):
        d_rows = 1152  # pad contraction: extra ones-row in x picks up b from W
        WT_full = np.zeros((d_rows, 4 * K), np.float32)
        WT_full[:D_IN] = W.T
        WT_full[D_IN] = b
    else:
        d_rows = D_IN
        WT_full = np.ascontiguousarray(W.T)
    DT = d_rows // P
    w8u = np.ascontiguousarray(
        _q8(WT_full[:, 0:K], S_W).reshape(DT, P, K).transpose(1, 0, 2))
    w8vz = np.ascontiguousarray(
        _q8(WT_full[:, K:3 * K], S_W).reshape(DT, P, 2 * K).transpose(1, 0, 2))
    wtt = np.ascontiguousarray(
        WT_full[:, 3 * K:].astype(BF16).reshape(DT, P, K).transpose(1, 0, 2))
    nc = _get_nc(d_rows)
    in_maps = []
    for c in range(NCORES):
        xs = x[c * NLOC:(c + 1) * NLOC]
        if d_rows == D_IN:
            xTs = np.ascontiguousarray(xs.T)
        else:
            xTs = np.zeros((d_rows, NLOC), np.float32)
            xTs[:D_IN] = xs.T
            xTs[D_IN] = 1.0
        xb_bf = xTs.astype(BF16)
        x8f = _q8(xb_bf.astype(np.float32), S_X)
        # pack into block-contiguous layouts so every device load is one
        # [128 x 8KB] 2D DMA (the row-gather pattern runs at ~83GB/s)
        xb_p = np.ascontiguousarray(
            xb_bf.reshape(DT, P, NB, IB).transpose(2, 1, 0, 3))
        x8_p = np.ascontiguousarray(
            x8f.reshape(DT, P, NLOC // 1024, 1024).transpose(2, 1, 0, 3))
        in_maps.append({"x8": x8_p, "xb": xb_p, "w8u": w8u, "w8vz": w8vz, "wtt": wtt})
    res = run_bass_kernel_spmd(
        nc, in_maps, list(range(NCORES)),
        trace=trace, **({"trace_cores": trace_cores} if trace_cores else {}),
    )
    full = np.concatenate([res.results[c]["out"] for c in range(NCORES)], axis=0)
    return full, res


def kernel(x, W, b):
    full, _ = _run(x, W, b)
    return full
